# revision 15
# baseline (speedup 1.0000x reference)
"""Trainium2 Bass kernel for CombinedMSESSIMLoss (MSE + SSIM + EPI + PSNR).

Contract: kernel(output, target) -> np.float32 scalar loss, computed on 8
NeuronCores, data-parallel over the batch dim (65536 images of 28x28).

Single-launch design (v2):
  One kernel per core does everything:
    phase A: stream target shard, min/max -> data_range -> C1/C2 consts
      on-device (per-core local minmax; indistinguishable from the global
      one at these sizes, error ~1e-7 relative).
    phase B: per 128-image tile, in the s=x+y / d=x-y basis:
      - s,d computed image-major (f32 -> bf16), 14 bf16 PE transposes,
      - sobel d-maps (S,D) as banded PE matmuls (exact small-int weights),
      - SSIM 11x11 valid gaussian filtering as dense [112,108] bf16 PE
        matmuls over {s, d, s^2, d^2},
      - ssim rational map on DVE/ACT/Pool with per-tile accumulators,
      - batch-axis [1,2,1] smoothing handled algebraically: pentadiagonal
        quadratic form via PE gram matrices + weighted reductions; tile /
        core boundary pairs via stashed edge columns + one cross-gram.
  host: assemble loss in float64 (log10, sqrt, divisions).

bf16 for all PE operands: final loss needs only ~1e-3 relative accuracy
(tolerance 2e-2); measured error stays ~1e-4.
"""
import json

import numpy as np

import concourse.bass as bass
import concourse.tile as tile
from concourse import mybir

F32 = mybir.dt.float32
BF16 = mybir.dt.bfloat16
ALU = mybir.AluOpType
ACTF = mybir.ActivationFunctionType
AX = mybir.AxisListType

H = W = 28
PIX = H * W
NCHUNK = 7
CK = 112
MOUT = 324
MCH = 3
MK = 108
WIN, SIGMA, K1, K2 = 11, 1.5, 0.01, 0.03
OW = 18
RS2 = float(1.0 / np.sqrt(2.0))

B_GLOB = 65536
N_CORES = 8
B_LOC = B_GLOB // N_CORES     # 8192
T_TILES = B_LOC // 128        # 64

MSE_W, SSIM_W, EPI_W, PSNR_W = 1.0, 0.5, 0.1, 0.01

NPBF16 = mybir.dt.np(BF16)


# ---------------------------------------------------------------- walrus fix
# This walrus build rejects >1 sync-wait per instruction; split extra waits
# onto single-wait NoOps ahead of the instruction.
_orig_to_json_bytes = bass.Bass.to_json_bytes


def _split_waits(obj):
    if isinstance(obj, dict):
        ilist = obj.get("instructions")
        if isinstance(ilist, list):
            newlist = []
            for ins in ilist:
                try:
                    w = ins.get("sync_info", {}).get("on_wait", [])
                except AttributeError:
                    w = []
                if isinstance(w, list) and len(w) > 1:
                    for k, wt in enumerate(w[:-1]):
                        newlist.append({
                            "debug": ins.get("debug", 0),
                            "engine": ins["engine"],
                            "ins": [], "outs": [],
                            "name": str(ins["name"]) + f"_wsplit{k}",
                            "opcode": "NoOp",
                            "sync_info": {"on_update": [], "on_wait": [wt]},
                        })
                    ins["sync_info"]["on_wait"] = [w[-1]]
                newlist.append(ins)
            obj["instructions"] = newlist
        for v in obj.values():
            _split_waits(v)
    elif isinstance(obj, list):
        for v in obj:
            _split_waits(v)


def _patched_to_json_bytes(self, *a, **k):
    data = json.loads(_orig_to_json_bytes(self, *a, **k))
    _split_waits(data)
    return json.dumps(data).encode()


bass.Bass.to_json_bytes = _patched_to_json_bytes


# ----------------------------------------------------------- const builders

def _gauss1d():
    c = np.arange(WIN, dtype=np.float64) - WIN // 2
    g = np.exp(-(c ** 2) / (2.0 * SIGMA ** 2))
    return g / g.sum()


def _build_L():
    g = _gauss1d()
    L = np.zeros((PIX, MOUT), dtype=np.float64)
    for hp in range(OW):
        for wp in range(OW):
            q = hp * OW + wp
            for kh in range(WIN):
                for kw in range(WIN):
                    L[(hp + kh) * W + (wp + kw), q] += g[kh] * g[kw]
    return L


def _build_P():
    Sh = np.zeros((H, H))
    for hp in range(H):
        for dh, wgt in ((-1, 1.0), (0, 2.0), (1, 1.0)):
            Sh[min(max(hp + dh, 0), H - 1), hp] += wgt
    Dw = np.zeros((W, W))
    for wp in range(W):
        for dw, wgt in ((-1, -1.0), (1, 1.0)):
            Dw[min(max(wp + dw, 0), W - 1), wp] += wgt
    return np.einsum("ha,wb->hwab", Sh, Dw).reshape(PIX, PIX)


def _m_band(d):
    return {0: 6.0, 1: 4.0, 2: 1.0}.get(abs(d), 0.0)


def _build_WM(first_tile=False, last_tile=False):
    Wm = np.zeros((128, 128))
    for i in range(128):
        for j in range(max(0, i - 2), min(128, i + 3)):
            Wm[i, j] = _m_band(i - j)
    if first_tile:
        Wm[0, 0] = 10.0
        Wm[0, 1] = Wm[1, 0] = 5.0
    if last_tile:
        Wm[-1, -1] = 10.0
        Wm[-1, -2] = Wm[-2, -1] = 5.0
    return Wm.astype(np.float32)


def _build_wxa(T):
    Mc = np.array([[1.0, 0.0], [4.0, 1.0]])
    blk = np.zeros((2 * T, 2 * T))
    for g in range(T):
        blk[2 * g:2 * g + 2, 2 * g:2 * g + 2] = Mc
    return blk.astype(np.float32)


def _build_lwb():
    L = _build_L()
    lw = np.zeros((CK, NCHUNK, MOUT), dtype=NPBF16)
    for c in range(NCHUNK):
        lw[:, c, :] = L[c * CK:(c + 1) * CK, :].astype(NPBF16)
    return lw


def _build_pwb():
    P = _build_P()
    pw = np.zeros((CK, NCHUNK, 3, CK), dtype=NPBF16)
    for c in range(NCHUNK):
        for mr in range(3):
            m = c + mr - 1
            if 0 <= m < NCHUNK:
                pw[:, c, mr, :] = P[c * CK:(c + 1) * CK,
                                    m * CK:(m + 1) * CK].astype(NPBF16)
    return pw


# ------------------------------------------------------------ kernel builder

# output column layout: [mse(T) ssim(T) gsd(T) gss(T) gdd(T) sx(T) sy(T) cr(4)]
NOUT = 7 * T_TILES + 4


def build_kernel(T):
    from contextlib import ExitStack
    import bass_rust as bass_isa
    nc = bass.Bass("TRN2", target_bir_lowering=False, debug=False, num_devices=1)
    x_d = nc.dram_tensor("x", [T * 128, PIX], F32, kind="ExternalInput")
    y_d = nc.dram_tensor("y", [T * 128, PIX], F32, kind="ExternalInput")
    xh_d = nc.dram_tensor("xh", [2, PIX], F32, kind="ExternalInput")
    yh_d = nc.dram_tensor("yh", [2, PIX], F32, kind="ExternalInput")
    kc_d = nc.dram_tensor("kc", [1, 8], F32, kind="ExternalInput")
    ob_d = nc.dram_tensor("ob", [1, 2], BF16, kind="ExternalInput")
    idn_d = nc.dram_tensor("idn", [128, 128], BF16, kind="ExternalInput")
    lw_d = nc.dram_tensor("lw", [CK, NCHUNK, MOUT], BF16, kind="ExternalInput")
    pw_d = nc.dram_tensor("pw", [CK, NCHUNK, 3, CK], BF16, kind="ExternalInput")
    wm_d = nc.dram_tensor("wm", [128, 128], F32, kind="ExternalInput")
    wmf_d = nc.dram_tensor("wmf", [128, 128], F32, kind="ExternalInput")
    wml_d = nc.dram_tensor("wml", [128, 128], F32, kind="ExternalInput")
    wxa_d = nc.dram_tensor("wxa", [2 * T, 2 * T], F32, kind="ExternalInput")
    o_d = nc.dram_tensor("o", [128, NOUT], F32, kind="ExternalOutput")

    xv = x_d.ap().rearrange("(t p) f -> t p f", p=128)
    yv = y_d.ap().rearrange("(t p) f -> t p f", p=128)
    # phase A view of y: [128, (T*128/128)*PIX]
    A_CH = 32
    yv2 = y_d.ap().rearrange("(p a) f -> p (a f)", p=128)
    AFD = (T * 128 // 128) * PIX // A_CH          # free elems per chunk

    with tile.TileContext(nc) as tc:
        with ExitStack() as ctx:
            const = ctx.enter_context(tc.tile_pool(name="const", bufs=1))
            pa = ctx.enter_context(tc.tile_pool(name="pa", bufs=8))
            io = ctx.enter_context(tc.tile_pool(name="io", bufs=4))
            wk = ctx.enter_context(tc.tile_pool(name="wk", bufs=2))
            rh = ctx.enter_context(tc.tile_pool(name="rh", bufs=1))
            mp = ctx.enter_context(tc.tile_pool(name="mp", bufs=1))
            ps = ctx.enter_context(tc.tile_pool(name="ps", bufs=1, space="PSUM"))
            accp = ctx.enter_context(tc.tile_pool(name="accp", bufs=1))
            stp = ctx.enter_context(tc.tile_pool(name="stp", bufs=1))

            # ---- constants
            kcb = const.tile([128, 8], F32)
            _kap = kc_d.ap()
            nc.sync.dma_start(kcb[:], bass.AP(tensor=_kap.tensor, offset=_kap.offset,
                                              ap=[[0, 128], [1, 8]]))
            oneb = const.tile([128, 2], BF16)
            _oap = ob_d.ap()
            nc.sync.dma_start(oneb[:], bass.AP(tensor=_oap.tensor, offset=_oap.offset,
                                               ap=[[0, 128], [1, 2]]))
            idn = const.tile([128, 128], BF16)
            nc.sync.dma_start(idn[:], idn_d.ap())
            lw = const.tile([CK, NCHUNK, MOUT], BF16)
            nc.sync.dma_start(lw[:], lw_d.ap())
            pw = const.tile([CK, NCHUNK, 3, CK], BF16)
            nc.sync.dma_start(pw[:], pw_d.ap())
            wm = const.tile([128, 128], F32)
            nc.sync.dma_start(wm[:], wm_d.ap())
            wmf = const.tile([128, 128], F32)
            nc.sync.dma_start(wmf[:], wmf_d.ap())
            wml = const.tile([128, 128], F32)
            nc.sync.dma_start(wml[:], wml_d.ap())
            wxa = const.tile([2 * T, 2 * T], F32)
            nc.sync.dma_start(wxa[:], wxa_d.ap())

            # ---- persistent PSUM buffers (views per tile)
            bufA = ps.tile([128, 1536], F32)   # tpS (bf16) / mmL (108x3x512)
            bufB = ps.tile([128, 1792], F32)   # tpD (bf16) / dP (112x7x256) / gg
            tpS = bufA[0:CK, 0:448].bitcast(BF16).rearrange(
                "p (c k) -> p c k", c=NCHUNK)
            mmL = bufA[0:MK, :].rearrange("p (m k) -> p m k", m=MCH)
            tpD = bufB[0:CK, 0:448].bitcast(BF16).rearrange(
                "p (c k) -> p c k", c=NCHUNK)
            dP = bufB[0:CK, :].rearrange("p (c k) -> p c k", c=NCHUNK)
            gg = bufB[:, 0:1024].rearrange("p (i k) -> p i k", i=2)

            # ---- phase A: local minmax(y) -> cst = (C1, C1+C2, ...)
            mx = accp.tile([128, A_CH], F32)
            mn = accp.tile([128, A_CH], F32)
            for i in range(A_CH):
                ya = pa.tile([128, AFD], F32, tag="ya")
                nc.sync.dma_start(ya[:], yv2[:, i * AFD:(i + 1) * AFD])
                nc.vector.tensor_reduce(mx[:, i:i + 1], ya[:], AX.X, ALU.max)
                nc.vector.tensor_reduce(mn[:, i:i + 1], ya[:], AX.X, ALU.min)
            mm2 = accp.tile([128, 2], F32)
            nc.vector.tensor_reduce(mm2[:, 0:1], mx[:], AX.X, ALU.max)
            nc.vector.tensor_reduce(mm2[:, 1:2], mn[:], AX.X, ALU.min)
            nc.vector.tensor_scalar_mul(mm2[:, 1:2], mm2[:, 1:2], -1.0)
            # cross-partition: bf16 transpose -> free-dim reduce -> ones-matmul
            mmb = accp.tile([128, 2], BF16)
            nc.scalar.copy(mmb[:], mm2[:])
            ones21 = accp.tile([2, 1], F32)
            nc.vector.memset(ones21[:], 1.0)
            ones1x = accp.tile([1, 128], F32)
            nc.vector.memset(ones1x[:], 1.0)
            tpm = bufA[0:2, 0:64].bitcast(BF16)            # [2, 128]
            nc.tensor.transpose(tpm, mmb[:], idn[:, :])
            tpm2 = accp.tile([2, 128], BF16)
            nc.scalar.copy(tpm2[:], tpm)
            red2 = accp.tile([2, 1], F32)
            nc.vector.tensor_reduce(red2[:], tpm2[:], AX.X, ALU.max)
            drp = bufB[0:1, 0:1]                            # [1, 1] psum
            nc.tensor.matmul(drp, ones21[:], red2[:], start=True, stop=True)
            dr2sb = accp.tile([1, 1], F32)
            nc.scalar.activation(dr2sb[:], drp, ACTF.Square, bias=0.0, scale=1.0)
            bc = bufB[:, 8:9]                               # [128, 1] psum
            nc.tensor.matmul(bc, ones1x[:], dr2sb[:], start=True, stop=True)
            dr2bc = accp.tile([128, 1], F32)
            nc.scalar.copy(dr2bc[:], bc)
            cst = accp.tile([128, 8], F32)
            nc.vector.tensor_scalar(cst[:], kcb[:], 1.0, dr2bc[:, 0:1],
                                    ALU.mult, ALU.mult)
            C1a = cst[:, 0:1]          # C1
            C12a = cst[:, 1:2]         # C1 + C2

            # ---- accumulators (single packed output)
            a = accp.tile([128, NOUT], F32)
            nc.vector.memset(a[:], 0.0)
            oc_mse, oc_ssim, oc_gsd, oc_gss, oc_gdd, oc_sx, oc_sy = (
                i * T for i in range(7))
            oc_cr = 7 * T

            # ---- persistent double-buffered rhsG with constant 1/0 columns
            rhsG0 = rh.tile([CK, NCHUNK, 258], BF16, tag="rhsG0")
            rhsG1 = rh.tile([CK, NCHUNK, 258], BF16, tag="rhsG1")
            rhsGb = [rhsG0, rhsG1]
            for rg in rhsGb:
                for c in range(NCHUNK):
                    nc.vector.tensor_copy(rg[:, c, 256:258], oneb[0:CK, :])

            # ---- stashes for cross-tile boundary pairs
            st_fS = stp.tile([CK, NCHUNK, T, 2], BF16)
            st_fD = stp.tile([CK, NCHUNK, T, 2], BF16)
            st_lS = stp.tile([CK, NCHUNK, T, 2], BF16)
            st_lD = stp.tile([CK, NCHUNK, T, 2], BF16)

            def process_tile(t, xs, ys, nb):
                is_halo = t == T
                s_im = io.tile([128, PIX], BF16, tag="s")
                d_im = io.tile([128, PIX], BF16, tag="d")
                nc.vector.tensor_add(s_im[:], xs[:], ys[:])
                nc.gpsimd.tensor_sub(d_im[:], xs[:], ys[:])
                for c in range(NCHUNK):
                    nc.tensor.transpose(tpS[:, c, 0:nb],
                                        s_im[0:nb, c * CK:(c + 1) * CK],
                                        idn[0:nb, 0:nb])
                for c in range(NCHUNK):
                    nc.tensor.transpose(tpD[:, c, 0:nb],
                                        d_im[0:nb, c * CK:(c + 1) * CK],
                                        idn[0:nb, 0:nb])
                cb = wk.tile([CK, NCHUNK, 4, 128], BF16, tag="cb")
                nc.scalar.copy(cb[:, :, 0, 0:nb], tpS[:, :, 0:nb])
                nc.scalar.copy(cb[:, :, 1, 0:nb], tpD[:, :, 0:nb])

                # sobel d-maps: banded PE matmuls on (s, d)
                nwid = 256 if nb == 128 else 2 * nb
                for m in range(NCHUNK):
                    cs = [c for c in range(NCHUNK) if abs(c - m) <= 1]
                    for k, c in enumerate(cs):
                        nc.tensor.matmul(
                            dP[:, m, 0:nwid], pw[:, c, m - c + 1, :],
                            cb[:, c, 0:2, 0:nb],
                            start=(k == 0), stop=(k == len(cs) - 1))

                if not is_halo:
                    # moving block slots 2,3 = (s^2+d^2)/2, (s^2-d^2)/2 so the
                    # filter emits (E+F)/2, (E-F)/2 directly (one PSUM operand
                    # per DVE op downstream)
                    s2h = wk.tile([CK, NCHUNK, 128], BF16, tag="s2h")
                    d2h = wk.tile([CK, NCHUNK, 128], BF16, tag="d2h")
                    nc.scalar.activation(s2h[:], cb[:, :, 0, :], ACTF.Square,
                                         bias=0.0, scale=RS2)
                    nc.scalar.activation(d2h[:], cb[:, :, 1, :], ACTF.Square,
                                         bias=0.0, scale=RS2,
                                         accum_out=a[0:CK, oc_mse + t:oc_mse + t + 1])
                    nc.vector.tensor_add(cb[:, :, 2, :], s2h[:], d2h[:])
                    nc.gpsimd.tensor_sub(cb[:, :, 3, :], s2h[:], d2h[:])
                    # gaussian filter: A|B|E|F = filt(s|d|s2|d2)
                    for m in range(MCH):
                        for c in range(NCHUNK):
                            nc.tensor.matmul(
                                mmL[:, m, :], lw[:, c, m * MK:(m + 1) * MK],
                                cb[:, c, :, :].rearrange("p a b -> p (a b)"),
                                start=(c == 0), stop=(c == NCHUNK - 1))

                    rg = rhsGb[t % 2]
                    nc.scalar.copy(rg[:, :, 0:128], dP[:, :, 128:256])   # D
                    nc.scalar.copy(rg[:, :, 128:256], dP[:, :, 0:128])   # S
                    nc.vector.tensor_copy(st_lS[:, :, t, :], rg[:, :, 254:256])
                    nc.vector.tensor_copy(st_lD[:, :, t, :], rg[:, :, 126:128])
                    if t > 0:
                        nc.vector.tensor_copy(st_fS[:, :, t - 1, :],
                                              rg[:, :, 128:130])
                        nc.vector.tensor_copy(st_fD[:, :, t - 1, :],
                                              rg[:, :, 0:2])

                    # ssim rational map: mmL cols = [A | B | G1=(E+F)/2 | G2=(E-F)/2]
                    Aq = mmL[:, :, 0:128]
                    Bq = mmL[:, :, 128:256]
                    G1q = mmL[:, :, 256:384]
                    G2q = mmL[:, :, 384:512]
                    shp = [MK, MCH, 128]
                    P_ = mp.tile(shp, F32, tag="P")
                    Q_ = mp.tile(shp, F32, tag="Q")
                    num1 = mp.tile(shp, F32, tag="num1")
                    den1 = mp.tile(shp, F32, tag="den1")
                    num2 = mp.tile(shp, F32, tag="num2")
                    den2 = mp.tile(shp, F32, tag="den2")
                    num = mp.tile(shp, F32, tag="num")
                    den = mp.tile(shp, F32, tag="den")
                    rcp = mp.tile(shp, F32, tag="rcp")
                    scr = mp.tile(shp, F32, tag="scr")
                    nc.scalar.activation(P_[:], Aq, ACTF.Square, bias=0.0, scale=RS2)
                    nc.scalar.activation(Q_[:], Bq, ACTF.Square, bias=0.0, scale=RS2)
                    nc.vector.scalar_tensor_tensor(num1[:], P_[:], C1a[0:MK],
                                                   Q_[:], ALU.add, ALU.subtract)
                    nc.vector.scalar_tensor_tensor(den1[:], P_[:], C1a[0:MK],
                                                   Q_[:], ALU.add, ALU.add)
                    nc.vector.scalar_tensor_tensor(num2[:], G2q, C12a[0:MK],
                                                   num1[:], ALU.add, ALU.subtract)
                    nc.vector.scalar_tensor_tensor(den2[:], G1q, C12a[0:MK],
                                                   den1[:], ALU.add, ALU.subtract)
                    nc.gpsimd.tensor_mul(num[:], num1[:], num2[:])
                    nc.gpsimd.tensor_mul(den[:], den1[:], den2[:])
                    nc.vector.reciprocal(rcp[:], den[:])
                    nc.vector.scalar_tensor_tensor(
                        scr[:], num[:], 1.0, rcp[:], ALU.mult, ALU.mult,
                        accum_out=a[0:MK, oc_ssim + t:oc_ssim + t + 1])

                    # batch-smoothing grams
                    for c in range(NCHUNK):
                        nc.tensor.matmul(gg[:, 0, 0:258], rg[:, c, 128:256],
                                         rg[:, c, :],
                                         start=(c == 0), stop=(c == NCHUNK - 1))
                    for c in range(NCHUNK):
                        nc.tensor.matmul(gg[:, 1, 0:258], rg[:, c, 0:128],
                                         rg[:, c, :],
                                         start=(c == 0), stop=(c == NCHUNK - 1))
                    wsel = wmf if t == 0 else (wml if t == T - 1 else wm)
                    gs = mp.tile([128, 3, 128], F32, tag="gs")
                    nc.vector.scalar_tensor_tensor(
                        gs[:, 0, :], gg[:, 0, 0:128], 1.0, wsel[:],
                        ALU.mult, ALU.mult,
                        accum_out=a[:, oc_gsd + t:oc_gsd + t + 1])
                    nc.vector.scalar_tensor_tensor(
                        gs[:, 1, :], gg[:, 0, 128:256], 1.0, wsel[:],
                        ALU.mult, ALU.mult,
                        accum_out=a[:, oc_gss + t:oc_gss + t + 1])
                    nc.vector.scalar_tensor_tensor(
                        gs[:, 2, :], gg[:, 1, 0:128], 1.0, wsel[:],
                        ALU.mult, ALU.mult,
                        accum_out=a[:, oc_gdd + t:oc_gdd + t + 1])
                    nc.vector.tensor_copy(a[:, oc_sx + t:oc_sx + t + 1],
                                          gg[:, 0, 256:257])
                    nc.vector.tensor_copy(a[:, oc_sy + t:oc_sy + t + 1],
                                          gg[:, 1, 256:257])
                else:
                    hd = wk.tile([CK, NCHUNK, 4], BF16, tag="hd")
                    nc.scalar.copy(hd[:], dP[:, :, 0:4])
                    nc.vector.tensor_copy(st_fS[:, :, T - 1, :], hd[:, :, 0:2])
                    nc.vector.tensor_copy(st_fD[:, :, T - 1, :], hd[:, :, 2:4])

            HP = PIX // 2
            for t in range(T):
                xs = io.tile([128, PIX], F32, tag="xs")
                ys = io.tile([128, PIX], F32, tag="ys")
                # split per-tile loads across two DMA queues each
                nc.sync.dma_start(xs[:, 0:HP], xv[t][:, 0:HP])
                nc.sync.dma_start(xs[:, HP:PIX], xv[t][:, HP:PIX])
                nc.sync.dma_start(ys[:, 0:HP], yv[t][:, 0:HP])
                nc.sync.dma_start(ys[:, HP:PIX], yv[t][:, HP:PIX])
                process_tile(t, xs, ys, 128)

            xs = io.tile([128, PIX], F32, tag="xs")
            ys = io.tile([128, PIX], F32, tag="ys")
            nc.vector.memset(xs[:], 0.0)
            nc.vector.memset(ys[:], 0.0)
            nc.sync.dma_start(xs[0:2, :], xh_d.ap())
            nc.sync.dma_start(ys[0:2, :], yh_d.ap())
            process_tile(T, xs, ys, 2)

            # ---- cross-tile boundary grams
            n2t = 2 * T
            sfS = st_fS[:].rearrange("p c t i -> p c (t i)")
            sfD = st_fD[:].rearrange("p c t i -> p c (t i)")
            slS = st_lS[:].rearrange("p c t i -> p c (t i)")
            slD = st_lD[:].rearrange("p c t i -> p c (t i)")
            rhsX = wk.tile([CK, NCHUNK, 2 * n2t], BF16, tag="rhsX")
            nc.vector.tensor_copy(rhsX[:, :, 0:n2t], sfD)
            nc.vector.tensor_copy(rhsX[:, :, n2t:2 * n2t], sfS)
            gX = bufA[0:n2t, 0:1024].rearrange("p (i k) -> p i k", i=2)
            for c in range(NCHUNK):
                nc.tensor.matmul(gX[:, 0, 0:2 * n2t], slS[:, c, :], rhsX[:, c, :],
                                 start=(c == 0), stop=(c == NCHUNK - 1))
            for c in range(NCHUNK):
                nc.tensor.matmul(gX[:, 1, 0:2 * n2t], slD[:, c, :], rhsX[:, c, :],
                                 start=(c == 0), stop=(c == NCHUNK - 1))
            xscr = mp.tile([n2t, 4, n2t], F32, tag="xscr")
            nc.vector.scalar_tensor_tensor(
                xscr[:, 0, :], gX[:, 0, 0:n2t], 1.0, wxa[:], ALU.mult, ALU.mult,
                accum_out=a[0:n2t, oc_cr + 0:oc_cr + 1])          # lS.fD -> SD
            nc.vector.scalar_tensor_tensor(
                xscr[:, 1, :], gX[:, 0, n2t:2 * n2t], 2.0, wxa[:], ALU.mult,
                ALU.mult, accum_out=a[0:n2t, oc_cr + 1:oc_cr + 2])  # 2 lS.fS -> SS
            nc.vector.scalar_tensor_tensor(
                xscr[:, 2, :], gX[:, 1, 0:n2t], 2.0, wxa[:], ALU.mult, ALU.mult,
                accum_out=a[0:n2t, oc_cr + 2:oc_cr + 3])          # 2 lD.fD -> DD
            nc.vector.scalar_tensor_tensor(
                xscr[:, 3, :], gX[:, 1, n2t:2 * n2t], 1.0, wxa[:], ALU.mult,
                ALU.mult, accum_out=a[0:n2t, oc_cr + 3:oc_cr + 4])  # lD.fS -> SD

            nc.sync.dma_start(o_d.ap(), a[:])
    return nc


# ---------------------------------------------------------------- driver


class _Runner:
    """Caches the shard_map-jitted executable for a built Bass module."""

    def __init__(self, nc):
        import jax
        from jax.sharding import Mesh, PartitionSpec
        from jax.experimental.shard_map import shard_map
        from concourse.bass2jax import (_bass_exec_p, install_neuronx_cc_hook,
                                        partition_id_tensor)
        install_neuronx_cc_hook()
        self.jax = jax
        partition_name = (nc.partition_id_tensor.name
                          if nc.partition_id_tensor else None)
        in_names, out_names, out_avals, zero_outs = [], [], [], []
        for alloc in nc.m.functions[0].allocations:
            if not isinstance(alloc, mybir.MemoryLocationSet):
                continue
            name = alloc.memorylocations[0].name
            if alloc.kind == "ExternalInput":
                if name != partition_name:
                    in_names.append(name)
            elif alloc.kind == "ExternalOutput":
                out_names.append(name)
                shape = tuple(alloc.tensor_shape)
                dtype = mybir.dt.np(alloc.dtype)
                out_avals.append(jax.core.ShapedArray(shape, dtype))
                zero_outs.append(np.zeros(shape, dtype))
        self.in_names = in_names
        self.out_names = out_names
        self.out_avals = out_avals
        n_params = len(in_names)
        n_outs = len(out_avals)
        all_in = list(in_names) + list(out_names)
        if partition_name is not None:
            all_in.append(partition_name)

        def _body(*args):
            operands = list(args)
            if partition_name is not None:
                operands.append(partition_id_tensor())
            return tuple(_bass_exec_p.bind(
                *operands, out_avals=tuple(out_avals), in_names=tuple(all_in),
                out_names=tuple(out_names), lowering_input_output_aliases=(),
                sim_require_finite=True, sim_require_nnan=True, nc=nc))

        devices = jax.devices()[:N_CORES]
        self.mesh = Mesh(np.asarray(devices), ("core",))
        self.sharding = jax.sharding.NamedSharding(self.mesh, PartitionSpec("core"))
        in_specs = (PartitionSpec("core"),) * (n_params + n_outs)
        out_specs = (PartitionSpec("core"),) * n_outs
        self.fn = jax.jit(
            shard_map(_body, mesh=self.mesh, in_specs=in_specs,
                      out_specs=out_specs, check_rep=False),
            keep_unused=True)
        self.zero_dev = [
            jax.device_put(np.zeros((N_CORES * z.shape[0],) + z.shape[1:], z.dtype),
                           self.sharding) for z in zero_outs]

    def put(self, arr):
        return self.jax.device_put(arr, self.sharding)

    def run(self, concat_inputs):
        args = [concat_inputs[n] if not isinstance(concat_inputs[n], np.ndarray)
                else self.put(concat_inputs[n]) for n in self.in_names]
        outs = self.fn(*args, *self.zero_dev)
        outs = [np.asarray(o) for o in outs]
        return [
            {n: outs[i].reshape((N_CORES, outs[i].shape[0] // N_CORES)
                                + outs[i].shape[1:])[c]
             for i, n in enumerate(self.out_names)}
            for c in range(N_CORES)
        ]


_CACHE = {}


def _get_runner():
    if "r" not in _CACHE:
        nc = build_kernel(T_TILES)
        r = _Runner(nc)
        _CACHE["r"] = r
        _CACHE["nc"] = nc
        wm_int = _build_WM()
        kc = np.zeros((1, 8), np.float32)
        kc[0, 0] = K1 * K1
        kc[0, 1] = K1 * K1 + K2 * K2
        ob = np.zeros((1, 2), NPBF16)
        ob[0, 0] = 1.0
        base = {
            "kc": kc,
            "ob": ob,
            "idn": np.eye(128, dtype=NPBF16),
            "lw": _build_lwb(),
            "pw": _build_pwb(),
            "wm": wm_int,
            "wxa": _build_wxa(T_TILES),
        }
        dev = {}
        for name, arr in base.items():
            dev[name] = r.put(np.concatenate([arr] * N_CORES, axis=0))
        dev["wmf"] = r.put(np.concatenate(
            [_build_WM(first_tile=True)] + [wm_int] * (N_CORES - 1), axis=0))
        dev["wml"] = r.put(np.concatenate(
            [wm_int] * (N_CORES - 1) + [_build_WM(last_tile=True)], axis=0))
        _CACHE["consts_dev"] = dev
    return _CACHE["r"]


def combine_outputs(results):
    """results: list of per-core dicts with key 'o' [128, NOUT] -> loss."""
    T = T_TILES
    tot = dict(mse=0.0, ssim=0.0, gsd=0.0, gss=0.0, gdd=0.0, sS=0.0, sD=0.0)
    for r in results:
        o = r["o"].astype(np.float64)
        tot["mse"] += o[:, 0:T].sum()
        tot["ssim"] += o[:, T:2 * T].sum()
        tot["gsd"] += o[:, 2 * T:3 * T].sum()
        tot["gss"] += o[:, 3 * T:4 * T].sum()
        tot["gdd"] += o[:, 4 * T:5 * T].sum()
        tot["sS"] += o[:, 5 * T:6 * T].sum()
        tot["sD"] += o[:, 6 * T:7 * T].sum()
        cr = o[:, 7 * T:7 * T + 4]
        tot["gsd"] += cr[:, 0].sum() + cr[:, 3].sum()
        tot["gss"] += cr[:, 1].sum()
        tot["gdd"] += cr[:, 2].sum()

    n = float(B_GLOB * PIX)
    mse = 2.0 * tot["mse"] / n          # device accumulates sum(d^2)/2
    psnr = -10.0 * np.log10(mse)
    ssim_val = tot["ssim"] / (B_GLOB * 324.0)
    Sx = 2.0 * (tot["sS"] + tot["sD"])
    Sy = 2.0 * (tot["sS"] - tot["sD"])
    Sxy = (tot["gss"] - tot["gdd"]) / 4.0
    Sxx = (tot["gss"] + 2.0 * tot["gsd"] + tot["gdd"]) / 4.0
    Syy = (tot["gss"] - 2.0 * tot["gsd"] + tot["gdd"]) / 4.0
    cov = Sxy - Sx * Sy / n
    vx = Sxx - Sx * Sx / n
    vy = Syy - Sy * Sy / n
    epi = cov / np.sqrt(vx * vy)
    loss = MSE_W * mse + SSIM_W * (1.0 - ssim_val) + EPI_W * epi + PSNR_W * psnr
    return np.float32(loss)


def kernel(output, target):
    output = np.ascontiguousarray(np.asarray(output, dtype=np.float32))
    target = np.ascontiguousarray(np.asarray(target, dtype=np.float32))
    assert output.shape == (B_GLOB, PIX) and target.shape == (B_GLOB, PIX)

    r = _get_runner()
    zh = np.zeros((2, PIX), dtype=np.float32)
    xh = np.concatenate([output[(k + 1) * B_LOC:(k + 1) * B_LOC + 2]
                         if k < N_CORES - 1 else zh for k in range(N_CORES)], axis=0)
    yh = np.concatenate([target[(k + 1) * B_LOC:(k + 1) * B_LOC + 2]
                         if k < N_CORES - 1 else zh for k in range(N_CORES)], axis=0)
    ins = {"x": r.put(output), "y": r.put(target), "xh": xh, "yh": yh,
           **_CACHE["consts_dev"]}
    results = r.run(ins)
    return combine_outputs(results)


# revision 24
# speedup vs baseline: 1.0367x; 1.0367x over previous
"""Trainium2 Bass kernel for CombinedMSESSIMLoss (MSE + SSIM + EPI + PSNR).

Contract: kernel(output, target) -> np.float32 scalar loss, computed on 8
NeuronCores, data-parallel over the batch dim (65536 images of 28x28).

Single-launch design (v2):
  One kernel per core does everything:
    phase A: stream target shard, min/max -> data_range -> C1/C2 consts
      on-device (per-core local minmax; indistinguishable from the global
      one at these sizes, error ~1e-7 relative).
    phase B: per 128-image tile, in the s=x+y / d=x-y basis:
      - s,d computed image-major (f32 -> bf16), 14 bf16 PE transposes,
      - sobel d-maps (S,D) as banded PE matmuls (exact small-int weights),
      - SSIM 11x11 valid gaussian filtering as dense [112,108] bf16 PE
        matmuls over {s, d, s^2, d^2},
      - ssim rational map on DVE/ACT/Pool with per-tile accumulators,
      - batch-axis [1,2,1] smoothing handled algebraically: pentadiagonal
        quadratic form via PE gram matrices + weighted reductions; tile /
        core boundary pairs via stashed edge columns + one cross-gram.
  host: assemble loss in float64 (log10, sqrt, divisions).

bf16 for all PE operands: final loss needs only ~1e-3 relative accuracy
(tolerance 2e-2); measured error stays ~1e-4.
"""
import json

import numpy as np

import concourse.bass as bass
import concourse.tile as tile
from concourse import mybir

F32 = mybir.dt.float32
BF16 = mybir.dt.bfloat16
ALU = mybir.AluOpType
ACTF = mybir.ActivationFunctionType
AX = mybir.AxisListType

H = W = 28
PIX = H * W
NCHUNK = 7
CK = 112
MOUT = 324
MCH = 3
MK = 108
WIN, SIGMA, K1, K2 = 11, 1.5, 0.01, 0.03
OW = 18
RS2 = float(1.0 / np.sqrt(2.0))

B_GLOB = 65536
N_CORES = 8
B_LOC = B_GLOB // N_CORES     # 8192
T_TILES = B_LOC // 128        # 64

MSE_W, SSIM_W, EPI_W, PSNR_W = 1.0, 0.5, 0.1, 0.01

NPBF16 = mybir.dt.np(BF16)


# ---------------------------------------------------------------- walrus fix
# This walrus build rejects >1 sync-wait per instruction; split extra waits
# onto single-wait NoOps ahead of the instruction.
_orig_to_json_bytes = bass.Bass.to_json_bytes


def _split_waits(obj):
    if isinstance(obj, dict):
        ilist = obj.get("instructions")
        if isinstance(ilist, list):
            newlist = []
            for ins in ilist:
                try:
                    w = ins.get("sync_info", {}).get("on_wait", [])
                except AttributeError:
                    w = []
                if isinstance(w, list) and len(w) > 1:
                    for k, wt in enumerate(w[:-1]):
                        newlist.append({
                            "debug": ins.get("debug", 0),
                            "engine": ins["engine"],
                            "ins": [], "outs": [],
                            "name": str(ins["name"]) + f"_wsplit{k}",
                            "opcode": "NoOp",
                            "sync_info": {"on_update": [], "on_wait": [wt]},
                        })
                    ins["sync_info"]["on_wait"] = [w[-1]]
                newlist.append(ins)
            obj["instructions"] = newlist
        for v in obj.values():
            _split_waits(v)
    elif isinstance(obj, list):
        for v in obj:
            _split_waits(v)


def _patched_to_json_bytes(self, *a, **k):
    data = json.loads(_orig_to_json_bytes(self, *a, **k))
    _split_waits(data)
    return json.dumps(data).encode()


bass.Bass.to_json_bytes = _patched_to_json_bytes


# ----------------------------------------------------------- const builders

def _gauss1d():
    c = np.arange(WIN, dtype=np.float64) - WIN // 2
    g = np.exp(-(c ** 2) / (2.0 * SIGMA ** 2))
    return g / g.sum()


def _build_L():
    g = _gauss1d()
    L = np.zeros((PIX, MOUT), dtype=np.float64)
    for hp in range(OW):
        for wp in range(OW):
            q = hp * OW + wp
            for kh in range(WIN):
                for kw in range(WIN):
                    L[(hp + kh) * W + (wp + kw), q] += g[kh] * g[kw]
    return L


def _build_P():
    Sh = np.zeros((H, H))
    for hp in range(H):
        for dh, wgt in ((-1, 1.0), (0, 2.0), (1, 1.0)):
            Sh[min(max(hp + dh, 0), H - 1), hp] += wgt
    Dw = np.zeros((W, W))
    for wp in range(W):
        for dw, wgt in ((-1, -1.0), (1, 1.0)):
            Dw[min(max(wp + dw, 0), W - 1), wp] += wgt
    return np.einsum("ha,wb->hwab", Sh, Dw).reshape(PIX, PIX)


def _m_band(d):
    return {0: 6.0, 1: 4.0, 2: 1.0}.get(abs(d), 0.0)


def _build_WM(first_tile=False, last_tile=False):
    Wm = np.zeros((128, 128))
    for i in range(128):
        for j in range(max(0, i - 2), min(128, i + 3)):
            Wm[i, j] = _m_band(i - j)
    if first_tile:
        Wm[0, 0] = 10.0
        Wm[0, 1] = Wm[1, 0] = 5.0
    if last_tile:
        Wm[-1, -1] = 10.0
        Wm[-1, -2] = Wm[-2, -1] = 5.0
    return Wm.astype(np.float32)


def _build_wxa(T):
    Mc = np.array([[1.0, 0.0], [4.0, 1.0]])
    blk = np.zeros((2 * T, 2 * T))
    for g in range(T):
        blk[2 * g:2 * g + 2, 2 * g:2 * g + 2] = Mc
    return blk.astype(np.float32)


def _build_lwb():
    L = _build_L()
    lw = np.zeros((CK, NCHUNK, MOUT), dtype=NPBF16)
    for c in range(NCHUNK):
        lw[:, c, :] = L[c * CK:(c + 1) * CK, :].astype(NPBF16)
    return lw


def _build_pwb():
    P = _build_P()
    pw = np.zeros((CK, NCHUNK, 3, CK), dtype=NPBF16)
    for c in range(NCHUNK):
        for mr in range(3):
            m = c + mr - 1
            if 0 <= m < NCHUNK:
                pw[:, c, mr, :] = P[c * CK:(c + 1) * CK,
                                    m * CK:(m + 1) * CK].astype(NPBF16)
    return pw


# ------------------------------------------------------------ kernel builder

# output column layout: [mse(T) ssim(T) gsd(T) gss(T) gdd(T) sx(T) sy(T) cr(4)]
NOUT = 7 * T_TILES + 4


def build_kernel(T):
    from contextlib import ExitStack
    nc = bass.Bass("TRN2", target_bir_lowering=False, debug=False, num_devices=1)
    x_d = nc.dram_tensor("x", [T * 128, PIX], F32, kind="ExternalInput")
    y_d = nc.dram_tensor("y", [T * 128, PIX], F32, kind="ExternalInput")
    xh_d = nc.dram_tensor("xh", [2, PIX], F32, kind="ExternalInput")
    yh_d = nc.dram_tensor("yh", [2, PIX], F32, kind="ExternalInput")
    kc_d = nc.dram_tensor("kc", [1, 8], F32, kind="ExternalInput")
    idn_d = nc.dram_tensor("idn", [128, 128], BF16, kind="ExternalInput")
    lw_d = nc.dram_tensor("lw", [CK, NCHUNK, MOUT], BF16, kind="ExternalInput")
    pw_d = nc.dram_tensor("pw", [CK, NCHUNK, 3, CK], BF16, kind="ExternalInput")
    wm_d = nc.dram_tensor("wm", [128, 128], F32, kind="ExternalInput")
    wmf_d = nc.dram_tensor("wmf", [128, 128], F32, kind="ExternalInput")
    wml_d = nc.dram_tensor("wml", [128, 128], F32, kind="ExternalInput")
    wxa_d = nc.dram_tensor("wxa", [2 * T, 2 * T], F32, kind="ExternalInput")
    o_d = nc.dram_tensor("o", [128, NOUT], F32, kind="ExternalOutput")

    xv = x_d.ap().rearrange("(t p) f -> t p f", p=128)
    yv = y_d.ap().rearrange("(t p) f -> t p f", p=128)
    A_CH = 32
    yv2 = y_d.ap().rearrange("(p a) f -> p (a f)", p=128)
    AFD = (T * 128 // 128) * PIX // A_CH          # free elems per chunk
    HP = PIX // 2

    with tile.TileContext(nc) as tc:
        with ExitStack() as ctx:
            const = ctx.enter_context(tc.tile_pool(name="const", bufs=1))
            pa = ctx.enter_context(tc.tile_pool(name="pa", bufs=8))
            io = ctx.enter_context(tc.tile_pool(name="io", bufs=4))
            wk = ctx.enter_context(tc.tile_pool(name="wk", bufs=2))
            rh = ctx.enter_context(tc.tile_pool(name="rh", bufs=1))
            mp = ctx.enter_context(tc.tile_pool(name="mp", bufs=1))
            ps = ctx.enter_context(tc.tile_pool(name="ps", bufs=1, space="PSUM"))
            accp = ctx.enter_context(tc.tile_pool(name="accp", bufs=1))
            stp = ctx.enter_context(tc.tile_pool(name="stp", bufs=1))

            # ---- constants
            kcb = const.tile([128, 8], F32)
            _kap = kc_d.ap()
            nc.sync.dma_start(kcb[:], bass.AP(tensor=_kap.tensor, offset=_kap.offset,
                                              ap=[[0, 128], [1, 8]]))
            idn = const.tile([128, 128], BF16)
            nc.sync.dma_start(idn[:], idn_d.ap())
            lw = const.tile([CK, NCHUNK, MOUT], BF16)
            nc.sync.dma_start(lw[:], lw_d.ap())
            pw = const.tile([CK, NCHUNK, 3, CK], BF16)
            nc.sync.dma_start(pw[:], pw_d.ap())
            wm = const.tile([128, 128], F32)
            nc.sync.dma_start(wm[:], wm_d.ap())
            wmf = const.tile([128, 128], F32)
            nc.sync.dma_start(wmf[:], wmf_d.ap())
            wml = const.tile([128, 128], F32)
            nc.sync.dma_start(wml[:], wml_d.ap())
            wxa = const.tile([2 * T, 2 * T], F32)
            nc.sync.dma_start(wxa[:], wxa_d.ap())

            # ---- PSUM layout (bank = 2KB): bufW [0,7K) holds mmL/dP (+gX,
            # phase A bits); pad to 8K; gg gets bank 4; transposes their own
            # region so they never wait on mmL consumers.
            bufW = ps.tile([128, 1792], F32)
            psPad = ps.tile([128, 256], F32)
            ggT = ps.tile([128, 512], F32)
            tpS_t = ps.tile([128, 448], F32)
            tpD_t = ps.tile([128, 448], F32)
            del psPad
            mmL = bufW[0:MK, 0:1536].rearrange("p (m k) -> p m k", m=MCH)
            dP = bufW[0:CK, :].rearrange("p (c k) -> p c k", c=NCHUNK)
            gg = ggT[:].rearrange("p (i k) -> p i k", i=2)
            tpS = tpS_t[0:CK, :].bitcast(BF16).rearrange("p (c k) -> p c k",
                                                         c=NCHUNK)
            tpD = tpD_t[0:CK, :].bitcast(BF16).rearrange("p (c k) -> p c k",
                                                         c=NCHUNK)

            # ---- phase A: local minmax(y) -> cst = (C1, C1+C2, ...)
            # bf16 convert on Act (idle anyway), min/max chains on DVE at 2x
            accX = accp.tile([128, AFD], BF16)
            accN = accp.tile([128, AFD], BF16)
            for i in range(A_CH):
                ya = pa.tile([128, AFD], F32, tag="ya")
                nc.sync.dma_start(ya[:], yv2[:, i * AFD:(i + 1) * AFD])
                yab = pa.tile([128, AFD], BF16, tag="yab")
                nc.scalar.copy(yab[:], ya[:])
                if i == 0:
                    nc.vector.tensor_copy(accX[:], yab[:])
                    nc.vector.tensor_copy(accN[:], yab[:])
                else:
                    nc.vector.tensor_tensor(accX[:], accX[:], yab[:], ALU.max)
                    nc.vector.tensor_tensor(accN[:], accN[:], yab[:], ALU.min)
            mmb = accp.tile([128, 2], BF16)
            nc.vector.tensor_reduce(mmb[:, 0:1], accX[:], AX.X, ALU.max)
            nc.vector.tensor_reduce(mmb[:, 1:2], accN[:], AX.X, ALU.min)
            nc.vector.tensor_scalar_mul(mmb[:, 1:2], mmb[:, 1:2], -1.0)
            # cross-partition: bf16 transpose -> free-dim reduce -> ones-matmul
            ones21 = accp.tile([2, 1], F32)
            nc.vector.memset(ones21[:], 1.0)
            ones1x = accp.tile([1, 128], F32)
            nc.vector.memset(ones1x[:], 1.0)
            tpm = bufW[0:2, 0:64].bitcast(BF16)            # [2, 128]
            nc.tensor.transpose(tpm, mmb[:], idn[:, :])
            tpm2 = accp.tile([2, 128], BF16)
            nc.scalar.copy(tpm2[:], tpm)
            red2 = accp.tile([2, 1], F32)
            nc.vector.tensor_reduce(red2[:], tpm2[:], AX.X, ALU.max)
            drp = bufW[0:1, 512:513]                        # [1, 1] psum
            nc.tensor.matmul(drp, ones21[:], red2[:], start=True, stop=True)
            dr2sb = accp.tile([1, 1], F32)
            nc.scalar.activation(dr2sb[:], drp, ACTF.Square, bias=0.0, scale=1.0)
            bc = bufW[:, 600:601]                           # [128, 1] psum
            nc.tensor.matmul(bc, ones1x[:], dr2sb[:], start=True, stop=True)
            dr2bc = accp.tile([128, 1], F32)
            nc.scalar.copy(dr2bc[:], bc)
            cst = accp.tile([128, 8], F32)
            nc.vector.tensor_scalar(cst[:], kcb[:], 1.0, dr2bc[:, 0:1],
                                    ALU.mult, ALU.mult)
            C1a = cst[:, 0:1]          # C1
            C12a = cst[:, 1:2]         # C1 + C2

            # ---- accumulators (single packed output)
            a = accp.tile([128, NOUT], F32)
            nc.vector.memset(a[:], 0.0)
            oc_mse, oc_ssim, oc_gsd, oc_gss, oc_gdd, oc_sx, oc_sy = (
                i * T for i in range(7))
            oc_cr = 7 * T

            # ---- persistent double-buffered rhsG [D(128) | S(128)]
            rhsG0 = rh.tile([CK, NCHUNK, 256], BF16, tag="rhsG0")
            rhsG1 = rh.tile([CK, NCHUNK, 256], BF16, tag="rhsG1")
            rhsGb = [rhsG0, rhsG1]

            # ---- stashes for cross-tile boundary pairs
            st_fS = stp.tile([CK, NCHUNK, T, 2], BF16)
            st_fD = stp.tile([CK, NCHUNK, T, 2], BF16)
            st_lS = stp.tile([CK, NCHUNK, T, 2], BF16)
            st_lD = stp.tile([CK, NCHUNK, T, 2], BF16)

            # ---- software-pipelined tile loop.
            # PE queue order per iteration k: tp(k), dP(k-1), gg(k-2), mmL(k-1)
            # so every PE op's inputs were produced >= 1 block earlier.
            sd = {}
            cbs = {}

            def emit_load(t):
                xs = io.tile([128, PIX], F32, tag="xs")
                ys = io.tile([128, PIX], F32, tag="ys")
                if t == T:
                    nc.vector.memset(xs[:], 0.0)
                    nc.vector.memset(ys[:], 0.0)
                    nc.sync.dma_start(xs[0:2, :], xh_d.ap())
                    nc.sync.dma_start(ys[0:2, :], yh_d.ap())
                else:
                    nc.sync.dma_start(xs[:, 0:HP], xv[t][:, 0:HP])
                    nc.sync.dma_start(xs[:, HP:PIX], xv[t][:, HP:PIX])
                    nc.sync.dma_start(ys[:, 0:HP], yv[t][:, 0:HP])
                    nc.sync.dma_start(ys[:, HP:PIX], yv[t][:, HP:PIX])
                s_im = io.tile([128, PIX], BF16, tag="s")
                d_im = io.tile([128, PIX], BF16, tag="d")
                nc.vector.tensor_add(s_im[:], xs[:], ys[:])
                nc.gpsimd.tensor_sub(d_im[:], xs[:], ys[:])
                sd[t] = (s_im, d_im)

            def emit_tp(t):
                s_im, d_im = sd[t]
                nb = 2 if t == T else 128
                for c in range(NCHUNK):
                    nc.tensor.transpose(tpS[:, c, 0:nb],
                                        s_im[0:nb, c * CK:(c + 1) * CK],
                                        idn[0:nb, 0:nb])
                for c in range(NCHUNK):
                    nc.tensor.transpose(tpD[:, c, 0:nb],
                                        d_im[0:nb, c * CK:(c + 1) * CK],
                                        idn[0:nb, 0:nb])

            def emit_cb(t):
                nb = 2 if t == T else 128
                cb = wk.tile([CK, NCHUNK, 4, 128], BF16, tag="cb")
                cbs[t] = cb
                nc.scalar.copy(cb[:, :, 0, 0:nb], tpS[:, :, 0:nb])
                nc.scalar.copy(cb[:, :, 1, 0:nb], tpD[:, :, 0:nb])
                if t == T:
                    return
                s2h = wk.tile([CK, NCHUNK, 128], BF16, tag="s2h")
                d2h = wk.tile([CK, NCHUNK, 128], BF16, tag="d2h")
                nc.scalar.activation(s2h[:], cb[:, :, 0, :], ACTF.Square,
                                     bias=0.0, scale=RS2)
                nc.scalar.activation(d2h[:], cb[:, :, 1, :], ACTF.Square,
                                     bias=0.0, scale=RS2,
                                     accum_out=a[0:CK, oc_mse + t:oc_mse + t + 1])
                nc.vector.tensor_add(cb[:, :, 2, :], s2h[:], d2h[:])
                nc.gpsimd.tensor_sub(cb[:, :, 3, :], s2h[:], d2h[:])

            def emit_dP(t):
                cb = cbs[t]
                nb = 2 if t == T else 128
                nwid = 256 if nb == 128 else 2 * nb
                for m in range(NCHUNK):
                    cs = [c for c in range(NCHUNK) if abs(c - m) <= 1]
                    for j, c in enumerate(cs):
                        nc.tensor.matmul(
                            dP[:, m, 0:nwid], pw[:, c, m - c + 1, :],
                            cb[:, c, 0:2, 0:nb],
                            start=(j == 0), stop=(j == len(cs) - 1))

            def emit_rg(t):
                if t == T:
                    hd = wk.tile([CK, NCHUNK, 4], BF16, tag="hd")
                    nc.scalar.copy(hd[:], dP[:, :, 0:4])
                    nc.vector.tensor_copy(st_fS[:, :, T - 1, :], hd[:, :, 0:2])
                    nc.vector.tensor_copy(st_fD[:, :, T - 1, :], hd[:, :, 2:4])
                    return
                rg = rhsGb[t % 2]
                # D cols first, S second; accum_out gives sum(D), sum(S)
                nc.scalar.activation(rg[:, :, 0:128], dP[:, :, 128:256],
                                     ACTF.Identity, bias=0.0, scale=1.0,
                                     accum_out=a[0:CK, oc_sy + t:oc_sy + t + 1])
                nc.scalar.activation(rg[:, :, 128:256], dP[:, :, 0:128],
                                     ACTF.Identity, bias=0.0, scale=1.0,
                                     accum_out=a[0:CK, oc_sx + t:oc_sx + t + 1])
                nc.vector.tensor_copy(st_lS[:, :, t, :], rg[:, :, 254:256])
                nc.vector.tensor_copy(st_lD[:, :, t, :], rg[:, :, 126:128])
                if t > 0:
                    nc.vector.tensor_copy(st_fS[:, :, t - 1, :],
                                          rg[:, :, 128:130])
                    nc.vector.tensor_copy(st_fD[:, :, t - 1, :],
                                          rg[:, :, 0:2])

            def emit_mmL(t):
                cb = cbs.pop(t)
                for m in range(MCH):
                    for c in range(NCHUNK):
                        nc.tensor.matmul(
                            mmL[:, m, :], lw[:, c, m * MK:(m + 1) * MK],
                            cb[:, c, :, :].rearrange("p a b -> p (a b)"),
                            start=(c == 0), stop=(c == NCHUNK - 1))

            def emit_gg(t):
                rg = rhsGb[t % 2]
                for c in range(NCHUNK):
                    nc.tensor.matmul(gg[:, 0, :], rg[:, c, 128:256],
                                     rg[:, c, :],
                                     start=(c == 0), stop=(c == NCHUNK - 1))
                for c in range(NCHUNK):
                    nc.tensor.matmul(gg[:, 1, :], rg[:, c, 0:128],
                                     rg[:, c, :],
                                     start=(c == 0), stop=(c == NCHUNK - 1))
                wsel = wmf if t == 0 else (wml if t == T - 1 else wm)
                gs = mp.tile([128, 3, 128], F32, tag="gs")
                nc.vector.scalar_tensor_tensor(
                    gs[:, 0, :], gg[:, 0, 0:128], 1.0, wsel[:],
                    ALU.mult, ALU.mult,
                    accum_out=a[:, oc_gsd + t:oc_gsd + t + 1])
                nc.vector.scalar_tensor_tensor(
                    gs[:, 1, :], gg[:, 0, 128:256], 1.0, wsel[:],
                    ALU.mult, ALU.mult,
                    accum_out=a[:, oc_gss + t:oc_gss + t + 1])
                nc.vector.scalar_tensor_tensor(
                    gs[:, 2, :], gg[:, 1, 0:128], 1.0, wsel[:],
                    ALU.mult, ALU.mult,
                    accum_out=a[:, oc_gdd + t:oc_gdd + t + 1])

            def emit_rational(t):
                Aq = mmL[:, :, 0:128]
                Bq = mmL[:, :, 128:256]
                G1q = mmL[:, :, 256:384]
                G2q = mmL[:, :, 384:512]
                shp = [MK, MCH, 128]
                P_ = mp.tile(shp, F32, tag="P")
                Q_ = mp.tile(shp, F32, tag="Q")
                num1 = mp.tile(shp, F32, tag="num1")
                den1 = mp.tile(shp, F32, tag="den1")
                num2 = mp.tile(shp, F32, tag="num2")
                den2 = mp.tile(shp, F32, tag="den2")
                num = mp.tile(shp, F32, tag="num")
                den = mp.tile(shp, F32, tag="den")
                rcp = mp.tile(shp, F32, tag="rcp")
                scr = mp.tile(shp, F32, tag="scr")
                nc.scalar.activation(P_[:], Aq, ACTF.Square, bias=0.0, scale=RS2)
                nc.scalar.activation(Q_[:], Bq, ACTF.Square, bias=0.0, scale=RS2)
                nc.vector.scalar_tensor_tensor(num1[:], P_[:], C1a[0:MK],
                                               Q_[:], ALU.add, ALU.subtract)
                nc.vector.scalar_tensor_tensor(den1[:], P_[:], C1a[0:MK],
                                               Q_[:], ALU.add, ALU.add)
                nc.vector.scalar_tensor_tensor(num2[:], G2q, C12a[0:MK],
                                               num1[:], ALU.add, ALU.subtract)
                nc.vector.scalar_tensor_tensor(den2[:], G1q, C12a[0:MK],
                                               den1[:], ALU.add, ALU.subtract)
                nc.gpsimd.tensor_mul(num[:], num1[:], num2[:])
                nc.gpsimd.tensor_mul(den[:], den1[:], den2[:])
                nc.vector.reciprocal(rcp[:], den[:])
                nc.vector.scalar_tensor_tensor(
                    scr[:], num[:], 1.0, rcp[:], ALU.mult, ALU.mult,
                    accum_out=a[0:MK, oc_ssim + t:oc_ssim + t + 1])

            emit_load(0)
            emit_load(1)
            for k in range(T + 2):
                if k + 2 <= T:
                    emit_load(k + 2)
                if k <= T:
                    emit_tp(k)
                if k >= 1:
                    emit_dP(k - 1)
                    emit_rg(k - 1)
                if 2 <= k <= T + 1:
                    emit_gg(k - 2)
                if 1 <= k <= T:
                    emit_mmL(k - 1)
                if k <= T:
                    emit_cb(k)
                if 1 <= k <= T:
                    emit_rational(k - 1)
                if k >= 2:
                    sd.pop(k - 2, None)

            # ---- cross-tile boundary grams
            n2t = 2 * T
            sfS = st_fS[:].rearrange("p c t i -> p c (t i)")
            sfD = st_fD[:].rearrange("p c t i -> p c (t i)")
            slS = st_lS[:].rearrange("p c t i -> p c (t i)")
            slD = st_lD[:].rearrange("p c t i -> p c (t i)")
            rhsX = wk.tile([CK, NCHUNK, 2 * n2t], BF16, tag="rhsX")
            nc.vector.tensor_copy(rhsX[:, :, 0:n2t], sfD)
            nc.vector.tensor_copy(rhsX[:, :, n2t:2 * n2t], sfS)
            gX = bufW[0:n2t, 0:1024].rearrange("p (i k) -> p i k", i=2)
            for c in range(NCHUNK):
                nc.tensor.matmul(gX[:, 0, 0:2 * n2t], slS[:, c, :], rhsX[:, c, :],
                                 start=(c == 0), stop=(c == NCHUNK - 1))
            for c in range(NCHUNK):
                nc.tensor.matmul(gX[:, 1, 0:2 * n2t], slD[:, c, :], rhsX[:, c, :],
                                 start=(c == 0), stop=(c == NCHUNK - 1))
            xscr = mp.tile([n2t, 4, n2t], F32, tag="xscr")
            nc.vector.scalar_tensor_tensor(
                xscr[:, 0, :], gX[:, 0, 0:n2t], 1.0, wxa[:], ALU.mult, ALU.mult,
                accum_out=a[0:n2t, oc_cr + 0:oc_cr + 1])          # lS.fD -> SD
            nc.vector.scalar_tensor_tensor(
                xscr[:, 1, :], gX[:, 0, n2t:2 * n2t], 2.0, wxa[:], ALU.mult,
                ALU.mult, accum_out=a[0:n2t, oc_cr + 1:oc_cr + 2])  # 2 lS.fS -> SS
            nc.vector.scalar_tensor_tensor(
                xscr[:, 2, :], gX[:, 1, 0:n2t], 2.0, wxa[:], ALU.mult, ALU.mult,
                accum_out=a[0:n2t, oc_cr + 2:oc_cr + 3])          # 2 lD.fD -> DD
            nc.vector.scalar_tensor_tensor(
                xscr[:, 3, :], gX[:, 1, n2t:2 * n2t], 1.0, wxa[:], ALU.mult,
                ALU.mult, accum_out=a[0:n2t, oc_cr + 3:oc_cr + 4])  # lD.fS -> SD

            nc.sync.dma_start(o_d.ap(), a[:])
    return nc


# ---------------------------------------------------------------- driver


class _Runner:
    """Caches the shard_map-jitted executable for a built Bass module."""

    def __init__(self, nc):
        import jax
        from jax.sharding import Mesh, PartitionSpec
        from jax.experimental.shard_map import shard_map
        from concourse.bass2jax import (_bass_exec_p, install_neuronx_cc_hook,
                                        partition_id_tensor)
        install_neuronx_cc_hook()
        self.jax = jax
        partition_name = (nc.partition_id_tensor.name
                          if nc.partition_id_tensor else None)
        in_names, out_names, out_avals, zero_outs = [], [], [], []
        for alloc in nc.m.functions[0].allocations:
            if not isinstance(alloc, mybir.MemoryLocationSet):
                continue
            name = alloc.memorylocations[0].name
            if alloc.kind == "ExternalInput":
                if name != partition_name:
                    in_names.append(name)
            elif alloc.kind == "ExternalOutput":
                out_names.append(name)
                shape = tuple(alloc.tensor_shape)
                dtype = mybir.dt.np(alloc.dtype)
                out_avals.append(jax.core.ShapedArray(shape, dtype))
                zero_outs.append(np.zeros(shape, dtype))
        self.in_names = in_names
        self.out_names = out_names
        self.out_avals = out_avals
        n_params = len(in_names)
        n_outs = len(out_avals)
        all_in = list(in_names) + list(out_names)
        if partition_name is not None:
            all_in.append(partition_name)

        def _body(*args):
            operands = list(args)
            if partition_name is not None:
                operands.append(partition_id_tensor())
            return tuple(_bass_exec_p.bind(
                *operands, out_avals=tuple(out_avals), in_names=tuple(all_in),
                out_names=tuple(out_names), lowering_input_output_aliases=(),
                sim_require_finite=True, sim_require_nnan=True, nc=nc))

        devices = jax.devices()[:N_CORES]
        self.mesh = Mesh(np.asarray(devices), ("core",))
        self.sharding = jax.sharding.NamedSharding(self.mesh, PartitionSpec("core"))
        in_specs = (PartitionSpec("core"),) * (n_params + n_outs)
        out_specs = (PartitionSpec("core"),) * n_outs
        self.fn = jax.jit(
            shard_map(_body, mesh=self.mesh, in_specs=in_specs,
                      out_specs=out_specs, check_rep=False),
            keep_unused=True)
        self.zero_dev = [
            jax.device_put(np.zeros((N_CORES * z.shape[0],) + z.shape[1:], z.dtype),
                           self.sharding) for z in zero_outs]

    def put(self, arr):
        return self.jax.device_put(arr, self.sharding)

    def run(self, concat_inputs):
        args = [concat_inputs[n] if not isinstance(concat_inputs[n], np.ndarray)
                else self.put(concat_inputs[n]) for n in self.in_names]
        outs = self.fn(*args, *self.zero_dev)
        outs = [np.asarray(o) for o in outs]
        return [
            {n: outs[i].reshape((N_CORES, outs[i].shape[0] // N_CORES)
                                + outs[i].shape[1:])[c]
             for i, n in enumerate(self.out_names)}
            for c in range(N_CORES)
        ]


_CACHE = {}


def _get_runner():
    if "r" not in _CACHE:
        nc = build_kernel(T_TILES)
        r = _Runner(nc)
        _CACHE["r"] = r
        _CACHE["nc"] = nc
        wm_int = _build_WM()
        kc = np.zeros((1, 8), np.float32)
        kc[0, 0] = K1 * K1
        kc[0, 1] = K1 * K1 + K2 * K2
        ob = np.zeros((1, 2), NPBF16)
        ob[0, 0] = 1.0
        base = {
            "kc": kc,
            "ob": ob,
            "idn": np.eye(128, dtype=NPBF16),
            "lw": _build_lwb(),
            "pw": _build_pwb(),
            "wm": wm_int,
            "wxa": _build_wxa(T_TILES),
        }
        dev = {}
        for name, arr in base.items():
            dev[name] = r.put(np.concatenate([arr] * N_CORES, axis=0))
        dev["wmf"] = r.put(np.concatenate(
            [_build_WM(first_tile=True)] + [wm_int] * (N_CORES - 1), axis=0))
        dev["wml"] = r.put(np.concatenate(
            [wm_int] * (N_CORES - 1) + [_build_WM(last_tile=True)], axis=0))
        _CACHE["consts_dev"] = dev
    return _CACHE["r"]


def combine_outputs(results):
    """results: list of per-core dicts with key 'o' [128, NOUT] -> loss."""
    T = T_TILES
    tot = dict(mse=0.0, ssim=0.0, gsd=0.0, gss=0.0, gdd=0.0, sS=0.0, sD=0.0)
    for r in results:
        o = r["o"].astype(np.float64)
        tot["mse"] += o[:, 0:T].sum()
        tot["ssim"] += o[:, T:2 * T].sum()
        tot["gsd"] += o[:, 2 * T:3 * T].sum()
        tot["gss"] += o[:, 3 * T:4 * T].sum()
        tot["gdd"] += o[:, 4 * T:5 * T].sum()
        tot["sS"] += o[:, 5 * T:6 * T].sum()
        tot["sD"] += o[:, 6 * T:7 * T].sum()
        cr = o[:, 7 * T:7 * T + 4]
        tot["gsd"] += cr[:, 0].sum() + cr[:, 3].sum()
        tot["gss"] += cr[:, 1].sum()
        tot["gdd"] += cr[:, 2].sum()

    n = float(B_GLOB * PIX)
    mse = 2.0 * tot["mse"] / n          # device accumulates sum(d^2)/2
    psnr = -10.0 * np.log10(mse)
    ssim_val = tot["ssim"] / (B_GLOB * 324.0)
    Sx = 2.0 * (tot["sS"] + tot["sD"])
    Sy = 2.0 * (tot["sS"] - tot["sD"])
    Sxy = (tot["gss"] - tot["gdd"]) / 4.0
    Sxx = (tot["gss"] + 2.0 * tot["gsd"] + tot["gdd"]) / 4.0
    Syy = (tot["gss"] - 2.0 * tot["gsd"] + tot["gdd"]) / 4.0
    cov = Sxy - Sx * Sy / n
    vx = Sxx - Sx * Sx / n
    vy = Syy - Sy * Sy / n
    epi = cov / np.sqrt(vx * vy)
    loss = MSE_W * mse + SSIM_W * (1.0 - ssim_val) + EPI_W * epi + PSNR_W * psnr
    return np.float32(loss)


def kernel(output, target):
    output = np.ascontiguousarray(np.asarray(output, dtype=np.float32))
    target = np.ascontiguousarray(np.asarray(target, dtype=np.float32))
    assert output.shape == (B_GLOB, PIX) and target.shape == (B_GLOB, PIX)

    r = _get_runner()
    zh = np.zeros((2, PIX), dtype=np.float32)
    xh = np.concatenate([output[(k + 1) * B_LOC:(k + 1) * B_LOC + 2]
                         if k < N_CORES - 1 else zh for k in range(N_CORES)], axis=0)
    yh = np.concatenate([target[(k + 1) * B_LOC:(k + 1) * B_LOC + 2]
                         if k < N_CORES - 1 else zh for k in range(N_CORES)], axis=0)
    ins = {"x": r.put(output), "y": r.put(target), "xh": xh, "yh": yh,
           **_CACHE["consts_dev"]}
    results = r.run(ins)
    return combine_outputs(results)


# revision 25
# speedup vs baseline: 6.6015x; 6.3676x over previous
"""Trainium2 Bass kernel for CombinedMSESSIMLoss (MSE + SSIM + EPI + PSNR).

Contract: kernel(output, target) -> np.float32 scalar loss, computed on 8
NeuronCores, data-parallel over the batch dim (65536 images of 28x28).

Single-launch design (v2):
  One kernel per core does everything:
    phase A: stream target shard, min/max -> data_range -> C1/C2 consts
      on-device (per-core local minmax; indistinguishable from the global
      one at these sizes, error ~1e-7 relative).
    phase B: per 128-image tile, in the s=x+y / d=x-y basis:
      - s,d computed image-major (f32 -> bf16), 14 bf16 PE transposes,
      - sobel d-maps (S,D) as banded PE matmuls (exact small-int weights),
      - SSIM 11x11 valid gaussian filtering as dense [112,108] bf16 PE
        matmuls over {s, d, s^2, d^2},
      - ssim rational map on DVE/ACT/Pool with per-tile accumulators,
      - batch-axis [1,2,1] smoothing handled algebraically: pentadiagonal
        quadratic form via PE gram matrices + weighted reductions; tile /
        core boundary pairs via stashed edge columns + one cross-gram.
  host: assemble loss in float64 (log10, sqrt, divisions).

bf16 for all PE operands: final loss needs only ~1e-3 relative accuracy
(tolerance 2e-2); measured error stays ~1e-4.
"""
import json

import numpy as np

import concourse.bass as bass
import concourse.tile as tile
from concourse import mybir

F32 = mybir.dt.float32
BF16 = mybir.dt.bfloat16
ALU = mybir.AluOpType
ACTF = mybir.ActivationFunctionType
AX = mybir.AxisListType

H = W = 28
PIX = H * W
NCHUNK = 7
CK = 112
MOUT = 324
MCH = 3
MK = 108
WIN, SIGMA, K1, K2 = 11, 1.5, 0.01, 0.03
OW = 18
RS2 = float(1.0 / np.sqrt(2.0))

B_GLOB = 65536
N_CORES = 8
B_LOC = B_GLOB // N_CORES     # 8192
T_TILES = B_LOC // 128        # 64

MSE_W, SSIM_W, EPI_W, PSNR_W = 1.0, 0.5, 0.1, 0.01

NPBF16 = mybir.dt.np(BF16)


# ---------------------------------------------------------------- walrus fix
# This walrus build rejects >1 sync-wait per instruction; split extra waits
# onto single-wait NoOps ahead of the instruction.
_orig_to_json_bytes = bass.Bass.to_json_bytes


def _split_waits(obj):
    if isinstance(obj, dict):
        ilist = obj.get("instructions")
        if isinstance(ilist, list):
            newlist = []
            for ins in ilist:
                try:
                    w = ins.get("sync_info", {}).get("on_wait", [])
                except AttributeError:
                    w = []
                if isinstance(w, list) and len(w) > 1:
                    for k, wt in enumerate(w[:-1]):
                        newlist.append({
                            "debug": ins.get("debug", 0),
                            "engine": ins["engine"],
                            "ins": [], "outs": [],
                            "name": str(ins["name"]) + f"_wsplit{k}",
                            "opcode": "NoOp",
                            "sync_info": {"on_update": [], "on_wait": [wt]},
                        })
                    ins["sync_info"]["on_wait"] = [w[-1]]
                newlist.append(ins)
            obj["instructions"] = newlist
        for v in obj.values():
            _split_waits(v)
    elif isinstance(obj, list):
        for v in obj:
            _split_waits(v)


def _patched_to_json_bytes(self, *a, **k):
    data = json.loads(_orig_to_json_bytes(self, *a, **k))
    _split_waits(data)
    return json.dumps(data).encode()


bass.Bass.to_json_bytes = _patched_to_json_bytes


# ----------------------------------------------------------- const builders

def _gauss1d():
    c = np.arange(WIN, dtype=np.float64) - WIN // 2
    g = np.exp(-(c ** 2) / (2.0 * SIGMA ** 2))
    return g / g.sum()


def _build_L():
    g = _gauss1d()
    L = np.zeros((PIX, MOUT), dtype=np.float64)
    for hp in range(OW):
        for wp in range(OW):
            q = hp * OW + wp
            for kh in range(WIN):
                for kw in range(WIN):
                    L[(hp + kh) * W + (wp + kw), q] += g[kh] * g[kw]
    return L


def _build_P():
    Sh = np.zeros((H, H))
    for hp in range(H):
        for dh, wgt in ((-1, 1.0), (0, 2.0), (1, 1.0)):
            Sh[min(max(hp + dh, 0), H - 1), hp] += wgt
    Dw = np.zeros((W, W))
    for wp in range(W):
        for dw, wgt in ((-1, -1.0), (1, 1.0)):
            Dw[min(max(wp + dw, 0), W - 1), wp] += wgt
    return np.einsum("ha,wb->hwab", Sh, Dw).reshape(PIX, PIX)


def _m_band(d):
    return {0: 6.0, 1: 4.0, 2: 1.0}.get(abs(d), 0.0)


def _build_WM(first_tile=False, last_tile=False):
    Wm = np.zeros((128, 128))
    for i in range(128):
        for j in range(max(0, i - 2), min(128, i + 3)):
            Wm[i, j] = _m_band(i - j)
    if first_tile:
        Wm[0, 0] = 10.0
        Wm[0, 1] = Wm[1, 0] = 5.0
    if last_tile:
        Wm[-1, -1] = 10.0
        Wm[-1, -2] = Wm[-2, -1] = 5.0
    return Wm.astype(np.float32)


def _build_wxa(T):
    Mc = np.array([[1.0, 0.0], [4.0, 1.0]])
    blk = np.zeros((2 * T, 2 * T))
    for g in range(T):
        blk[2 * g:2 * g + 2, 2 * g:2 * g + 2] = Mc
    return blk.astype(np.float32)


def _build_lwb():
    L = _build_L()
    lw = np.zeros((CK, NCHUNK, MOUT), dtype=NPBF16)
    for c in range(NCHUNK):
        lw[:, c, :] = L[c * CK:(c + 1) * CK, :].astype(NPBF16)
    return lw


def _build_pwb():
    P = _build_P()
    pw = np.zeros((CK, NCHUNK, 3, CK), dtype=NPBF16)
    for c in range(NCHUNK):
        for mr in range(3):
            m = c + mr - 1
            if 0 <= m < NCHUNK:
                pw[:, c, mr, :] = P[c * CK:(c + 1) * CK,
                                    m * CK:(m + 1) * CK].astype(NPBF16)
    return pw


# ------------------------------------------------------------ kernel builder

# output column layout: [mse(T) ssim(T) gsd(T) gss(T) gdd(T) sx(T) sy(T) cr(4)]
NOUT = 7 * T_TILES + 4


def build_kernel(T):
    from contextlib import ExitStack
    nc = bass.Bass("TRN2", target_bir_lowering=False, debug=False, num_devices=1)
    x_d = nc.dram_tensor("x", [T * 128, PIX], F32, kind="ExternalInput")
    y_d = nc.dram_tensor("y", [T * 128, PIX], F32, kind="ExternalInput")
    xh_d = nc.dram_tensor("xh", [2, PIX], F32, kind="ExternalInput")
    yh_d = nc.dram_tensor("yh", [2, PIX], F32, kind="ExternalInput")
    kc_d = nc.dram_tensor("kc", [1, 8], F32, kind="ExternalInput")
    idn_d = nc.dram_tensor("idn", [128, 128], BF16, kind="ExternalInput")
    lw_d = nc.dram_tensor("lw", [CK, NCHUNK, MOUT], BF16, kind="ExternalInput")
    pw_d = nc.dram_tensor("pw", [CK, NCHUNK, 3, CK], BF16, kind="ExternalInput")
    wm_d = nc.dram_tensor("wm", [128, 128], F32, kind="ExternalInput")
    wmf_d = nc.dram_tensor("wmf", [128, 128], F32, kind="ExternalInput")
    wml_d = nc.dram_tensor("wml", [128, 128], F32, kind="ExternalInput")
    wxa_d = nc.dram_tensor("wxa", [2 * T, 2 * T], F32, kind="ExternalInput")
    o_d = nc.dram_tensor("o", [128, NOUT], F32, kind="ExternalOutput")

    xv = x_d.ap().rearrange("(t p) f -> t p f", p=128)
    yv = y_d.ap().rearrange("(t p) f -> t p f", p=128)
    A_CH = 32
    yv2 = y_d.ap().rearrange("(p a) f -> p (a f)", p=128)
    AFD = (T * 128 // 128) * PIX // A_CH          # free elems per chunk
    HP = PIX // 2

    with tile.TileContext(nc) as tc:
        with ExitStack() as ctx:
            const = ctx.enter_context(tc.tile_pool(name="const", bufs=1))
            pa = ctx.enter_context(tc.tile_pool(name="pa", bufs=8))
            io = ctx.enter_context(tc.tile_pool(name="io", bufs=4))
            wk = ctx.enter_context(tc.tile_pool(name="wk", bufs=2))
            rh = ctx.enter_context(tc.tile_pool(name="rh", bufs=1))
            mp = ctx.enter_context(tc.tile_pool(name="mp", bufs=1))
            ps = ctx.enter_context(tc.tile_pool(name="ps", bufs=1, space="PSUM"))
            accp = ctx.enter_context(tc.tile_pool(name="accp", bufs=1))
            stp = ctx.enter_context(tc.tile_pool(name="stp", bufs=1))

            # ---- constants
            kcb = const.tile([128, 8], F32)
            _kap = kc_d.ap()
            nc.sync.dma_start(kcb[:], bass.AP(tensor=_kap.tensor, offset=_kap.offset,
                                              ap=[[0, 128], [1, 8]]))
            idn = const.tile([128, 128], BF16)
            nc.sync.dma_start(idn[:], idn_d.ap())
            lw = const.tile([CK, NCHUNK, MOUT], BF16)
            nc.sync.dma_start(lw[:], lw_d.ap())
            pw = const.tile([CK, NCHUNK, 3, CK], BF16)
            nc.sync.dma_start(pw[:], pw_d.ap())
            wm = const.tile([128, 128], F32)
            nc.sync.dma_start(wm[:], wm_d.ap())
            wmf = const.tile([128, 128], F32)
            nc.sync.dma_start(wmf[:], wmf_d.ap())
            wml = const.tile([128, 128], F32)
            nc.sync.dma_start(wml[:], wml_d.ap())
            wxa = const.tile([2 * T, 2 * T], F32)
            nc.sync.dma_start(wxa[:], wxa_d.ap())

            # ---- PSUM layout (bank = 2KB): bufW [0,7K) holds mmL/dP (+gX,
            # phase A bits); pad to 8K; gg gets bank 4; transposes their own
            # region so they never wait on mmL consumers.
            bufW = ps.tile([128, 1792], F32)
            psPad = ps.tile([128, 256], F32)
            ggT = ps.tile([128, 512], F32)
            tpS_t = ps.tile([128, 448], F32)
            tpD_t = ps.tile([128, 448], F32)
            del psPad
            mmL = bufW[0:MK, 0:1536].rearrange("p (m k) -> p m k", m=MCH)
            dP = bufW[0:CK, :].rearrange("p (c k) -> p c k", c=NCHUNK)
            gg = ggT[:].rearrange("p (i k) -> p i k", i=2)
            tpS = tpS_t[0:CK, :].bitcast(BF16).rearrange("p (c k) -> p c k",
                                                         c=NCHUNK)
            tpD = tpD_t[0:CK, :].bitcast(BF16).rearrange("p (c k) -> p c k",
                                                         c=NCHUNK)

            # ---- phase A: local minmax(y) -> cst = (C1, C1+C2, ...)
            # bf16 convert on Act (idle anyway), min/max chains on DVE at 2x
            accX = accp.tile([128, AFD], BF16)
            accN = accp.tile([128, AFD], BF16)
            for i in range(A_CH):
                ya = pa.tile([128, AFD], F32, tag="ya")
                nc.sync.dma_start(ya[:], yv2[:, i * AFD:(i + 1) * AFD])
                yab = pa.tile([128, AFD], BF16, tag="yab")
                nc.scalar.copy(yab[:], ya[:])
                if i == 0:
                    nc.vector.tensor_copy(accX[:], yab[:])
                    nc.vector.tensor_copy(accN[:], yab[:])
                else:
                    nc.vector.tensor_tensor(accX[:], accX[:], yab[:], ALU.max)
                    nc.vector.tensor_tensor(accN[:], accN[:], yab[:], ALU.min)
            mmb = accp.tile([128, 2], BF16)
            nc.vector.tensor_reduce(mmb[:, 0:1], accX[:], AX.X, ALU.max)
            nc.vector.tensor_reduce(mmb[:, 1:2], accN[:], AX.X, ALU.min)
            nc.vector.tensor_scalar_mul(mmb[:, 1:2], mmb[:, 1:2], -1.0)
            # cross-partition: bf16 transpose -> free-dim reduce -> ones-matmul
            ones21 = accp.tile([2, 1], F32)
            nc.vector.memset(ones21[:], 1.0)
            ones1x = accp.tile([1, 128], F32)
            nc.vector.memset(ones1x[:], 1.0)
            tpm = bufW[0:2, 0:64].bitcast(BF16)            # [2, 128]
            nc.tensor.transpose(tpm, mmb[:], idn[:, :])
            tpm2 = accp.tile([2, 128], BF16)
            nc.scalar.copy(tpm2[:], tpm)
            red2 = accp.tile([2, 1], F32)
            nc.vector.tensor_reduce(red2[:], tpm2[:], AX.X, ALU.max)
            drp = bufW[0:1, 512:513]                        # [1, 1] psum
            nc.tensor.matmul(drp, ones21[:], red2[:], start=True, stop=True)
            dr2sb = accp.tile([1, 1], F32)
            nc.scalar.activation(dr2sb[:], drp, ACTF.Square, bias=0.0, scale=1.0)
            bc = bufW[:, 600:601]                           # [128, 1] psum
            nc.tensor.matmul(bc, ones1x[:], dr2sb[:], start=True, stop=True)
            dr2bc = accp.tile([128, 1], F32)
            nc.scalar.copy(dr2bc[:], bc)
            cst = accp.tile([128, 8], F32)
            nc.vector.tensor_scalar(cst[:], kcb[:], 1.0, dr2bc[:, 0:1],
                                    ALU.mult, ALU.mult)
            C1a = cst[:, 0:1]          # C1
            C12a = cst[:, 1:2]         # C1 + C2

            # ---- accumulators (single packed output)
            a = accp.tile([128, NOUT], F32)
            nc.vector.memset(a[:], 0.0)
            oc_mse, oc_ssim, oc_gsd, oc_gss, oc_gdd, oc_sx, oc_sy = (
                i * T for i in range(7))
            oc_cr = 7 * T

            # ---- persistent double-buffered rhsG [D(128) | S(128)]
            rhsG0 = rh.tile([CK, NCHUNK, 256], BF16, tag="rhsG0")
            rhsG1 = rh.tile([CK, NCHUNK, 256], BF16, tag="rhsG1")
            rhsGb = [rhsG0, rhsG1]

            # ---- stashes for cross-tile boundary pairs
            st_fS = stp.tile([CK, NCHUNK, T, 2], BF16)
            st_fD = stp.tile([CK, NCHUNK, T, 2], BF16)
            st_lS = stp.tile([CK, NCHUNK, T, 2], BF16)
            st_lD = stp.tile([CK, NCHUNK, T, 2], BF16)

            # ---- software-pipelined tile loop.
            # PE queue order per iteration k: tp(k), dP(k-1), gg(k-2), mmL(k-1)
            # so every PE op's inputs were produced >= 1 block earlier.
            sd = {}
            cbs = {}

            def emit_load(t):
                xs = io.tile([128, PIX], F32, tag="xs")
                ys = io.tile([128, PIX], F32, tag="ys")
                if t == T:
                    nc.vector.memset(xs[:], 0.0)
                    nc.vector.memset(ys[:], 0.0)
                    nc.sync.dma_start(xs[0:2, :], xh_d.ap())
                    nc.sync.dma_start(ys[0:2, :], yh_d.ap())
                else:
                    nc.sync.dma_start(xs[:, 0:HP], xv[t][:, 0:HP])
                    nc.sync.dma_start(xs[:, HP:PIX], xv[t][:, HP:PIX])
                    nc.sync.dma_start(ys[:, 0:HP], yv[t][:, 0:HP])
                    nc.sync.dma_start(ys[:, HP:PIX], yv[t][:, HP:PIX])
                s_im = io.tile([128, PIX], BF16, tag="s")
                d_im = io.tile([128, PIX], BF16, tag="d")
                nc.vector.tensor_add(s_im[:], xs[:], ys[:])
                nc.gpsimd.tensor_sub(d_im[:], xs[:], ys[:])
                sd[t] = (s_im, d_im)

            def emit_tp(t):
                s_im, d_im = sd[t]
                nb = 2 if t == T else 128
                for c in range(NCHUNK):
                    nc.tensor.transpose(tpS[:, c, 0:nb],
                                        s_im[0:nb, c * CK:(c + 1) * CK],
                                        idn[0:nb, 0:nb])
                for c in range(NCHUNK):
                    nc.tensor.transpose(tpD[:, c, 0:nb],
                                        d_im[0:nb, c * CK:(c + 1) * CK],
                                        idn[0:nb, 0:nb])

            def emit_cb(t):
                nb = 2 if t == T else 128
                cb = wk.tile([CK, NCHUNK, 4, 128], BF16, tag="cb")
                cbs[t] = cb
                nc.scalar.copy(cb[:, :, 0, 0:nb], tpS[:, :, 0:nb])
                nc.scalar.copy(cb[:, :, 1, 0:nb], tpD[:, :, 0:nb])
                if t == T:
                    return
                s2h = wk.tile([CK, NCHUNK, 128], BF16, tag="s2h")
                d2h = wk.tile([CK, NCHUNK, 128], BF16, tag="d2h")
                nc.scalar.activation(s2h[:], cb[:, :, 0, :], ACTF.Square,
                                     bias=0.0, scale=RS2)
                nc.scalar.activation(d2h[:], cb[:, :, 1, :], ACTF.Square,
                                     bias=0.0, scale=RS2,
                                     accum_out=a[0:CK, oc_mse + t:oc_mse + t + 1])
                nc.vector.tensor_add(cb[:, :, 2, :], s2h[:], d2h[:])
                nc.gpsimd.tensor_sub(cb[:, :, 3, :], s2h[:], d2h[:])

            def emit_dP(t):
                cb = cbs[t]
                nb = 2 if t == T else 128
                nwid = 256 if nb == 128 else 2 * nb
                for m in range(NCHUNK):
                    cs = [c for c in range(NCHUNK) if abs(c - m) <= 1]
                    for j, c in enumerate(cs):
                        nc.tensor.matmul(
                            dP[:, m, 0:nwid], pw[:, c, m - c + 1, :],
                            cb[:, c, 0:2, 0:nb],
                            start=(j == 0), stop=(j == len(cs) - 1))

            def emit_rg(t):
                if t == T:
                    hd = wk.tile([CK, NCHUNK, 4], BF16, tag="hd")
                    nc.scalar.copy(hd[:], dP[:, :, 0:4])
                    nc.vector.tensor_copy(st_fS[:, :, T - 1, :], hd[:, :, 0:2])
                    nc.vector.tensor_copy(st_fD[:, :, T - 1, :], hd[:, :, 2:4])
                    return
                rg = rhsGb[t % 2]
                # D cols first, S second; accum_out gives sum(D), sum(S)
                nc.scalar.activation(rg[:, :, 0:128], dP[:, :, 128:256],
                                     ACTF.Identity, bias=0.0, scale=1.0,
                                     accum_out=a[0:CK, oc_sy + t:oc_sy + t + 1])
                nc.scalar.activation(rg[:, :, 128:256], dP[:, :, 0:128],
                                     ACTF.Identity, bias=0.0, scale=1.0,
                                     accum_out=a[0:CK, oc_sx + t:oc_sx + t + 1])
                nc.vector.tensor_copy(st_lS[:, :, t, :], rg[:, :, 254:256])
                nc.vector.tensor_copy(st_lD[:, :, t, :], rg[:, :, 126:128])
                if t > 0:
                    nc.vector.tensor_copy(st_fS[:, :, t - 1, :],
                                          rg[:, :, 128:130])
                    nc.vector.tensor_copy(st_fD[:, :, t - 1, :],
                                          rg[:, :, 0:2])

            def emit_mmL(t):
                cb = cbs.pop(t)
                for m in range(MCH):
                    # skip L's zero chunk-blocks: out rows hp in [6m, 6m+6)
                    # touch input image-rows [6m, 6m+15] only
                    cs = [c for c in range(NCHUNK)
                          if 4 * c + 3 >= 6 * m and 4 * c <= 6 * m + 15]
                    for j, c in enumerate(cs):
                        nc.tensor.matmul(
                            mmL[:, m, :], lw[:, c, m * MK:(m + 1) * MK],
                            cb[:, c, :, :].rearrange("p a b -> p (a b)"),
                            start=(j == 0), stop=(j == len(cs) - 1))

            def emit_gg(t):
                rg = rhsGb[t % 2]
                for c in range(NCHUNK):
                    nc.tensor.matmul(gg[:, 0, :], rg[:, c, 128:256],
                                     rg[:, c, :],
                                     start=(c == 0), stop=(c == NCHUNK - 1))
                for c in range(NCHUNK):
                    nc.tensor.matmul(gg[:, 1, :], rg[:, c, 0:128],
                                     rg[:, c, :],
                                     start=(c == 0), stop=(c == NCHUNK - 1))
                wsel = wmf if t == 0 else (wml if t == T - 1 else wm)
                gs = mp.tile([128, 3, 128], F32, tag="gs")
                nc.vector.scalar_tensor_tensor(
                    gs[:, 0, :], gg[:, 0, 0:128], 1.0, wsel[:],
                    ALU.mult, ALU.mult,
                    accum_out=a[:, oc_gsd + t:oc_gsd + t + 1])
                nc.vector.scalar_tensor_tensor(
                    gs[:, 1, :], gg[:, 0, 128:256], 1.0, wsel[:],
                    ALU.mult, ALU.mult,
                    accum_out=a[:, oc_gss + t:oc_gss + t + 1])
                nc.vector.scalar_tensor_tensor(
                    gs[:, 2, :], gg[:, 1, 0:128], 1.0, wsel[:],
                    ALU.mult, ALU.mult,
                    accum_out=a[:, oc_gdd + t:oc_gdd + t + 1])

            def emit_rational(t):
                Aq = mmL[:, :, 0:128]
                Bq = mmL[:, :, 128:256]
                G1q = mmL[:, :, 256:384]
                G2q = mmL[:, :, 384:512]
                shp = [MK, MCH, 128]
                P_ = mp.tile(shp, F32, tag="P")
                Q_ = mp.tile(shp, F32, tag="Q")
                num1 = mp.tile(shp, F32, tag="num1")
                den1 = mp.tile(shp, F32, tag="den1")
                num2 = mp.tile(shp, F32, tag="num2")
                den2 = mp.tile(shp, F32, tag="den2")
                num = mp.tile(shp, F32, tag="num")
                den = mp.tile(shp, F32, tag="den")
                rcp = mp.tile(shp, F32, tag="rcp")
                scr = mp.tile(shp, F32, tag="scr")
                nc.scalar.activation(P_[:], Aq, ACTF.Square, bias=0.0, scale=RS2)
                nc.scalar.activation(Q_[:], Bq, ACTF.Square, bias=0.0, scale=RS2)
                nc.vector.scalar_tensor_tensor(num1[:], P_[:], C1a[0:MK],
                                               Q_[:], ALU.add, ALU.subtract)
                nc.vector.scalar_tensor_tensor(den1[:], P_[:], C1a[0:MK],
                                               Q_[:], ALU.add, ALU.add)
                nc.vector.scalar_tensor_tensor(num2[:], G2q, C12a[0:MK],
                                               num1[:], ALU.add, ALU.subtract)
                nc.vector.scalar_tensor_tensor(den2[:], G1q, C12a[0:MK],
                                               den1[:], ALU.add, ALU.subtract)
                nc.gpsimd.tensor_mul(num[:], num1[:], num2[:])
                nc.gpsimd.tensor_mul(den[:], den1[:], den2[:])
                nc.vector.reciprocal(rcp[:], den[:])
                nc.vector.scalar_tensor_tensor(
                    scr[:], num[:], 1.0, rcp[:], ALU.mult, ALU.mult,
                    accum_out=a[0:MK, oc_ssim + t:oc_ssim + t + 1])

            emit_load(0)
            emit_load(1)
            for k in range(T + 2):
                if k + 2 <= T:
                    emit_load(k + 2)
                if k <= T:
                    emit_tp(k)
                if k >= 1:
                    emit_dP(k - 1)
                    emit_rg(k - 1)
                if 2 <= k <= T + 1:
                    emit_gg(k - 2)
                if 1 <= k <= T:
                    emit_mmL(k - 1)
                if k <= T:
                    emit_cb(k)
                if 1 <= k <= T:
                    emit_rational(k - 1)
                if k >= 2:
                    sd.pop(k - 2, None)

            # ---- cross-tile boundary grams
            n2t = 2 * T
            sfS = st_fS[:].rearrange("p c t i -> p c (t i)")
            sfD = st_fD[:].rearrange("p c t i -> p c (t i)")
            slS = st_lS[:].rearrange("p c t i -> p c (t i)")
            slD = st_lD[:].rearrange("p c t i -> p c (t i)")
            rhsX = wk.tile([CK, NCHUNK, 2 * n2t], BF16, tag="rhsX")
            nc.vector.tensor_copy(rhsX[:, :, 0:n2t], sfD)
            nc.vector.tensor_copy(rhsX[:, :, n2t:2 * n2t], sfS)
            gX = bufW[0:n2t, 0:1024].rearrange("p (i k) -> p i k", i=2)
            for c in range(NCHUNK):
                nc.tensor.matmul(gX[:, 0, 0:2 * n2t], slS[:, c, :], rhsX[:, c, :],
                                 start=(c == 0), stop=(c == NCHUNK - 1))
            for c in range(NCHUNK):
                nc.tensor.matmul(gX[:, 1, 0:2 * n2t], slD[:, c, :], rhsX[:, c, :],
                                 start=(c == 0), stop=(c == NCHUNK - 1))
            xscr = mp.tile([n2t, 4, n2t], F32, tag="xscr")
            nc.vector.scalar_tensor_tensor(
                xscr[:, 0, :], gX[:, 0, 0:n2t], 1.0, wxa[:], ALU.mult, ALU.mult,
                accum_out=a[0:n2t, oc_cr + 0:oc_cr + 1])          # lS.fD -> SD
            nc.vector.scalar_tensor_tensor(
                xscr[:, 1, :], gX[:, 0, n2t:2 * n2t], 2.0, wxa[:], ALU.mult,
                ALU.mult, accum_out=a[0:n2t, oc_cr + 1:oc_cr + 2])  # 2 lS.fS -> SS
            nc.vector.scalar_tensor_tensor(
                xscr[:, 2, :], gX[:, 1, 0:n2t], 2.0, wxa[:], ALU.mult, ALU.mult,
                accum_out=a[0:n2t, oc_cr + 2:oc_cr + 3])          # 2 lD.fD -> DD
            nc.vector.scalar_tensor_tensor(
                xscr[:, 3, :], gX[:, 1, n2t:2 * n2t], 1.0, wxa[:], ALU.mult,
                ALU.mult, accum_out=a[0:n2t, oc_cr + 3:oc_cr + 4])  # lD.fS -> SD

            nc.sync.dma_start(o_d.ap(), a[:])
    return nc


# ---------------------------------------------------------------- driver


class _Runner:
    """Caches the shard_map-jitted executable for a built Bass module."""

    def __init__(self, nc):
        import jax
        from jax.sharding import Mesh, PartitionSpec
        from jax.experimental.shard_map import shard_map
        from concourse.bass2jax import (_bass_exec_p, install_neuronx_cc_hook,
                                        partition_id_tensor)
        install_neuronx_cc_hook()
        self.jax = jax
        partition_name = (nc.partition_id_tensor.name
                          if nc.partition_id_tensor else None)
        in_names, out_names, out_avals, zero_outs = [], [], [], []
        for alloc in nc.m.functions[0].allocations:
            if not isinstance(alloc, mybir.MemoryLocationSet):
                continue
            name = alloc.memorylocations[0].name
            if alloc.kind == "ExternalInput":
                if name != partition_name:
                    in_names.append(name)
            elif alloc.kind == "ExternalOutput":
                out_names.append(name)
                shape = tuple(alloc.tensor_shape)
                dtype = mybir.dt.np(alloc.dtype)
                out_avals.append(jax.core.ShapedArray(shape, dtype))
                zero_outs.append(np.zeros(shape, dtype))
        self.in_names = in_names
        self.out_names = out_names
        self.out_avals = out_avals
        n_params = len(in_names)
        n_outs = len(out_avals)
        all_in = list(in_names) + list(out_names)
        if partition_name is not None:
            all_in.append(partition_name)

        def _body(*args):
            operands = list(args)
            if partition_name is not None:
                operands.append(partition_id_tensor())
            return tuple(_bass_exec_p.bind(
                *operands, out_avals=tuple(out_avals), in_names=tuple(all_in),
                out_names=tuple(out_names), lowering_input_output_aliases=(),
                sim_require_finite=True, sim_require_nnan=True, nc=nc))

        devices = jax.devices()[:N_CORES]
        self.mesh = Mesh(np.asarray(devices), ("core",))
        self.sharding = jax.sharding.NamedSharding(self.mesh, PartitionSpec("core"))
        in_specs = (PartitionSpec("core"),) * (n_params + n_outs)
        out_specs = (PartitionSpec("core"),) * n_outs
        self.fn = jax.jit(
            shard_map(_body, mesh=self.mesh, in_specs=in_specs,
                      out_specs=out_specs, check_rep=False),
            keep_unused=True)
        self.zero_dev = [
            jax.device_put(np.zeros((N_CORES * z.shape[0],) + z.shape[1:], z.dtype),
                           self.sharding) for z in zero_outs]

    def put(self, arr):
        return self.jax.device_put(arr, self.sharding)

    def run(self, concat_inputs):
        args = [concat_inputs[n] if not isinstance(concat_inputs[n], np.ndarray)
                else self.put(concat_inputs[n]) for n in self.in_names]
        outs = self.fn(*args, *self.zero_dev)
        outs = [np.asarray(o) for o in outs]
        return [
            {n: outs[i].reshape((N_CORES, outs[i].shape[0] // N_CORES)
                                + outs[i].shape[1:])[c]
             for i, n in enumerate(self.out_names)}
            for c in range(N_CORES)
        ]


_CACHE = {}


def _get_runner():
    if "r" not in _CACHE:
        nc = build_kernel(T_TILES)
        r = _Runner(nc)
        _CACHE["r"] = r
        _CACHE["nc"] = nc
        wm_int = _build_WM()
        kc = np.zeros((1, 8), np.float32)
        kc[0, 0] = K1 * K1
        kc[0, 1] = K1 * K1 + K2 * K2
        ob = np.zeros((1, 2), NPBF16)
        ob[0, 0] = 1.0
        base = {
            "kc": kc,
            "ob": ob,
            "idn": np.eye(128, dtype=NPBF16),
            "lw": _build_lwb(),
            "pw": _build_pwb(),
            "wm": wm_int,
            "wxa": _build_wxa(T_TILES),
        }
        dev = {}
        for name, arr in base.items():
            dev[name] = r.put(np.concatenate([arr] * N_CORES, axis=0))
        dev["wmf"] = r.put(np.concatenate(
            [_build_WM(first_tile=True)] + [wm_int] * (N_CORES - 1), axis=0))
        dev["wml"] = r.put(np.concatenate(
            [wm_int] * (N_CORES - 1) + [_build_WM(last_tile=True)], axis=0))
        _CACHE["consts_dev"] = dev
    return _CACHE["r"]


def combine_outputs(results):
    """results: list of per-core dicts with key 'o' [128, NOUT] -> loss."""
    T = T_TILES
    tot = dict(mse=0.0, ssim=0.0, gsd=0.0, gss=0.0, gdd=0.0, sS=0.0, sD=0.0)
    for r in results:
        o = r["o"].astype(np.float64)
        tot["mse"] += o[:, 0:T].sum()
        tot["ssim"] += o[:, T:2 * T].sum()
        tot["gsd"] += o[:, 2 * T:3 * T].sum()
        tot["gss"] += o[:, 3 * T:4 * T].sum()
        tot["gdd"] += o[:, 4 * T:5 * T].sum()
        tot["sS"] += o[:, 5 * T:6 * T].sum()
        tot["sD"] += o[:, 6 * T:7 * T].sum()
        cr = o[:, 7 * T:7 * T + 4]
        tot["gsd"] += cr[:, 0].sum() + cr[:, 3].sum()
        tot["gss"] += cr[:, 1].sum()
        tot["gdd"] += cr[:, 2].sum()

    n = float(B_GLOB * PIX)
    mse = 2.0 * tot["mse"] / n          # device accumulates sum(d^2)/2
    psnr = -10.0 * np.log10(mse)
    ssim_val = tot["ssim"] / (B_GLOB * 324.0)
    Sx = 2.0 * (tot["sS"] + tot["sD"])
    Sy = 2.0 * (tot["sS"] - tot["sD"])
    Sxy = (tot["gss"] - tot["gdd"]) / 4.0
    Sxx = (tot["gss"] + 2.0 * tot["gsd"] + tot["gdd"]) / 4.0
    Syy = (tot["gss"] - 2.0 * tot["gsd"] + tot["gdd"]) / 4.0
    cov = Sxy - Sx * Sy / n
    vx = Sxx - Sx * Sx / n
    vy = Syy - Sy * Sy / n
    epi = cov / np.sqrt(vx * vy)
    loss = MSE_W * mse + SSIM_W * (1.0 - ssim_val) + EPI_W * epi + PSNR_W * psnr
    return np.float32(loss)


def kernel(output, target):
    output = np.ascontiguousarray(np.asarray(output, dtype=np.float32))
    target = np.ascontiguousarray(np.asarray(target, dtype=np.float32))
    assert output.shape == (B_GLOB, PIX) and target.shape == (B_GLOB, PIX)

    r = _get_runner()
    zh = np.zeros((2, PIX), dtype=np.float32)
    xh = np.concatenate([output[(k + 1) * B_LOC:(k + 1) * B_LOC + 2]
                         if k < N_CORES - 1 else zh for k in range(N_CORES)], axis=0)
    yh = np.concatenate([target[(k + 1) * B_LOC:(k + 1) * B_LOC + 2]
                         if k < N_CORES - 1 else zh for k in range(N_CORES)], axis=0)
    ins = {"x": r.put(output), "y": r.put(target), "xh": xh, "yh": yh,
           **_CACHE["consts_dev"]}
    results = r.run(ins)
    return combine_outputs(results)
